# revision 22
# baseline (speedup 1.0000x reference)
"""AGSM layer (gated sparse attention + FFN) on 8 TRN2 NeuronCores.

Data-parallel over the batch: B=8 batch elements -> 8 cores, one full
[N=2048, D=256] attention layer per core.  No collectives.

Host-side (numpy, O(D^2) marshalling): fold LN1/LN2 scales+biases into
the adjacent weight matrices, fold 1/sqrt(D) into Wq, fold proj_w into
Wv (W@(v@proj) == (W@v)@proj), precompute g^0.8.

Device (per core, reference semantics):
  xn  = (x - mean)/sqrt(var+eps)            per-row LN (affine folded away)
  qT,kT = wqk^T @ xn^T, gated by g^0.8 along the free axis
  A   = q'k'^T                               -> output #2 (f32)
  tau = xn @ tauw'
  mask= sigmoid(12(A - tau)) = 0.5 (1 + tanh(6(A - tau)))  (tanh shares the
        ACT table-set with exp; sigmoid does not; tanh kept in f32 because
        mask ~ 0 means tanh ~ -1 and bf16 would round the tail to exactly -1)
  P   = mask * 0.5*exp(A)    (softmax max-shift skipped: |A| <~ 1.5)
  att = (P @ v_ext) / rowsum(P)              v_ext = xn @ (Wv' proj) + bias row
  x2  = x + att
  out = x2 + W2^T gelu(W1'^T LN2(x2) + b1)   -> output #1
"""

import math
from contextlib import ExitStack

import ml_dtypes
import numpy as np

import concourse.bass as bass
import concourse.bacc as bacc
import concourse.tile as tile
import concourse.mybir as mybir
from concourse.bass_utils import run_bass_kernel_spmd
from concourse.masks import make_identity

B, N, D, DFF = 8, 2048, 256, 1024
P = 128
NT = N // P    # 16 query/key tiles
KT = D // P    # 2 d-tiles
FT = DFF // P  # 8 dff tiles
EPS = 1e-5
LN_HALF = math.log(0.5)

FP32 = mybir.dt.float32
BF16 = mybir.dt.bfloat16
AF = mybir.ActivationFunctionType
ALU = mybir.AluOpType


def _body(nc, tc, ctx, io):
    x_d, g8_d, wqk_d, wv2_d, batt_d, tauw_d, taub6_d, w1_d, b1c_d, w2_d, \
        f2b_d, out_d, attn_d = io

    consts = ctx.enter_context(tc.tile_pool(name="consts", bufs=1))
    stats = ctx.enter_context(tc.tile_pool(name="stats", bufs=6))
    work = ctx.enter_context(tc.tile_pool(name="work", bufs=2))
    xnp = ctx.enter_context(tc.tile_pool(name="xnp", bufs=3))
    ps_big = ctx.enter_context(tc.tile_pool(name="ps_big", bufs=3, space="PSUM"))
    ps_att = ctx.enter_context(tc.tile_pool(name="ps_att", bufs=2, space="PSUM"))
    ps_tp = ctx.enter_context(tc.tile_pool(name="ps_tp", bufs=2, space="PSUM"))

    # ---------------- constants / weights ----------------
    id_bf = consts.tile([P, P], BF16)
    make_identity(nc, id_bf)
    eps_t = consts.tile([P, 1], FP32)
    nc.vector.memset(eps_t, EPS)
    ones_row = consts.tile([1, P], BF16)
    nc.vector.memset(ones_row, 1.0)
    lnhalf_t = consts.tile([P, 1], FP32)
    nc.vector.memset(lnhalf_t, LN_HALF)

    wqk_l = consts.tile([P, KT, 4 * P], BF16)
    wv2_r = consts.tile([P, KT, D], BF16)
    w1_l = consts.tile([P, KT, DFF], BF16)
    for kt in range(KT):
        nc.sync.dma_start(out=wqk_l[:, kt, :], in_=wqk_d[kt * P:(kt + 1) * P, :])
        nc.sync.dma_start(out=wv2_r[:, kt, :], in_=wv2_d[kt * P:(kt + 1) * P, :])
        nc.sync.dma_start(out=w1_l[:, kt, :], in_=w1_d[kt * P:(kt + 1) * P, :])
    w2_r = consts.tile([P, FT, D], BF16)
    for mt in range(FT):
        nc.sync.dma_start(out=w2_r[:, mt, :], in_=w2_d[mt * P:(mt + 1) * P, :])
    tauw_c = consts.tile([P, KT], BF16)
    nc.sync.dma_start(out=tauw_c, in_=tauw_d.ap().rearrange("(k p) o -> p (k o)", p=P, k=KT))
    taub12 = consts.tile([P, 1], FP32)
    nc.sync.dma_start(out=taub12, in_=taub6_d.ap())
    b1c = consts.tile([P, FT], FP32)
    nc.sync.dma_start(out=b1c, in_=b1c_d.ap())
    bias_att = consts.tile([1, D], BF16)
    nc.sync.dma_start(out=bias_att, in_=batt_d.ap())
    f2b_row = consts.tile([1, D], BF16)
    nc.sync.dma_start(out=f2b_row, in_=f2b_d.ap())

    g_ap = g8_d.ap()
    g_bc = bass.AP(tensor=g_ap.tensor, offset=g_ap.offset, ap=[[0, P]] + list(g_ap.ap[1:]))
    gp = consts.tile([P, N], FP32)
    nc.sync.dma_start(out=gp, in_=g_bc)

    # ---------------- LN1 + transposed normalized input ----------------
    x_sb = consts.tile([P, NT, D], FP32)
    nc.sync.dma_start(out=x_sb, in_=x_d.ap().rearrange("(t p) d -> p t d", p=P))
    xnT = consts.tile([P, KT, N], BF16)
    for t in range(NT):
        st = stats.tile([P, 6], FP32)
        nc.vector.bn_stats(st, x_sb[:, t, :])
        mv = stats.tile([P, 2], FP32)
        nc.vector.bn_aggr(mv, st)
        std = stats.tile([P, 1], FP32)
        nc.scalar.activation(std, mv[:, 1:2], AF.Sqrt, bias=eps_t)
        r1 = stats.tile([P, 1], FP32)
        nc.vector.reciprocal(r1, std)
        xn = xnp.tile([P, D], BF16)
        nc.vector.tensor_scalar(xn, x_sb[:, t, :], scalar1=mv[:, 0:1], scalar2=r1,
                                op0=ALU.subtract, op1=ALU.mult)
        for kt in range(KT):
            tp = ps_tp.tile([P, P], BF16, tag="pt")
            nc.tensor.transpose(tp, xn[:, kt * P:(kt + 1) * P], id_bf)
            nc.vector.tensor_copy(xnT[:, kt, t * P:(t + 1) * P], tp)

    # ---------------- q/k (gated, transposed), v'' (proj-folded), tau ----------------
    qkT = consts.tile([P, 2 * KT, N], BF16)   # rows: q d-tiles 0..1, k d-tiles 2..3
    for mc in range(2 * KT):
        for c in range(4):
            ps = ps_big.tile([P, 512], FP32, tag="pb")
            for kt in range(KT):
                nc.tensor.matmul(ps, lhsT=wqk_l[:, kt, mc * P:(mc + 1) * P],
                                 rhs=xnT[:, kt, c * 512:(c + 1) * 512],
                                 start=(kt == 0), stop=(kt == KT - 1))
            nc.vector.tensor_mul(qkT[:, mc, c * 512:(c + 1) * 512], ps,
                                 gp[:, c * 512:(c + 1) * 512])

    v_ext = consts.tile([P, NT, D], BF16)
    for mt in range(NT):
        ps = ps_att.tile([P, D], FP32, tag="pa")
        for kt in range(KT):
            nc.tensor.matmul(ps, lhsT=xnT[:, kt, mt * P:(mt + 1) * P],
                             rhs=wv2_r[:, kt, :], start=(kt == 0), stop=False)
        nc.tensor.matmul(ps, lhsT=ones_row, rhs=bias_att, start=False, stop=True)
        nc.scalar.copy(v_ext[:, mt, :], ps)

    tanh6 = consts.tile([P, NT], FP32)
    for t in range(NT):
        ps = ps_att.tile([P, 1], FP32, tag="pa")
        for kt in range(KT):
            nc.tensor.matmul(ps, lhsT=xnT[:, kt, t * P:(t + 1) * P],
                             rhs=tauw_c[:, kt:kt + 1], start=(kt == 0), stop=(kt == KT - 1))
        nc.vector.scalar_tensor_tensor(out=tanh6[:, t:t + 1], in0=ps, scalar=-6.0,
                                       in1=taub12, op0=ALU.mult, op1=ALU.add)

    # ---------------- attention loop ----------------
    x2_sb = consts.tile([P, NT, D], FP32)
    attn_v = attn_d.ap().rearrange("(t p) m -> p t m", p=P)
    for t in range(NT):
        A_sb = work.tile([P, N], FP32, tag="A_sb")
        for c in range(4):
            ps = ps_big.tile([P, 512], FP32, tag="pb")
            for kt in range(KT):
                nc.tensor.matmul(ps, lhsT=qkT[:, kt, t * P:(t + 1) * P],
                                 rhs=qkT[:, KT + kt, c * 512:(c + 1) * 512],
                                 start=(kt == 0), stop=(kt == KT - 1))
            if c < 2:
                nc.scalar.copy(A_sb[:, c * 512:(c + 1) * 512], ps)
            else:
                nc.vector.tensor_copy(A_sb[:, c * 512:(c + 1) * 512], ps)
        nc.sync.dma_start(out=attn_v[:, t, :], in_=A_sb)
        # P = sigmoid(12(A-tau)) * exp(A); sigmoid via 0.5(1+tanh(.)) -- tanh
        # shares the ACT table-set with exp.  Tanh output stays f32 (near -1
        # the bf16 tail would round to exactly -1).
        E = work.tile([P, N], BF16, tag="E")
        nc.scalar.activation(E, A_sb, AF.Exp, bias=lnhalf_t)   # 0.5*exp(A)
        Th = work.tile([P, N], FP32, tag="Th")
        nc.scalar.activation(Th, A_sb, AF.Tanh, scale=6.0, bias=tanh6[:, t:t + 1])
        Pt = work.tile([P, N], BF16, tag="Pt")
        Tsum = stats.tile([P, 1], FP32)
        nc.vector.scalar_tensor_tensor(out=Pt, in0=Th, scalar=1.0, in1=E,
                                       op0=ALU.add, op1=ALU.mult, accum_out=Tsum)
        # The tanh LUT flushes to exactly -1 below ~-8, so a fully-masked row
        # sums to 0; guard the divide like the reference's +1e-12.
        Tsafe = stats.tile([P, 1], FP32)
        nc.vector.tensor_scalar_add(Tsafe, Tsum, 1e-30)
        PT = work.tile([P, NT, P], BF16, tag="PT")
        for mt in range(NT):
            nc.sync.dma_start(out=PT[:, mt, :], in_=Pt[:, mt * P:(mt + 1) * P],
                              transpose=True)
        rT = stats.tile([P, 1], FP32)
        nc.vector.reciprocal(rT, Tsafe)
        ps_o = ps_att.tile([P, D], FP32, tag="pa")
        for mt in range(NT):
            nc.tensor.matmul(ps_o, lhsT=PT[:, mt, :], rhs=v_ext[:, mt, :],
                             start=(mt == 0), stop=(mt == NT - 1))
        nc.vector.scalar_tensor_tensor(out=x2_sb[:, t, :], in0=ps_o, scalar=rT,
                                       in1=x_sb[:, t, :], op0=ALU.mult, op1=ALU.add)

    # ---------------- FFN ----------------
    out_v = out_d.ap().rearrange("(t p) d -> p t d", p=P)
    for c in range(4):
        h2nT = work.tile([P, KT, 512], BF16, tag="h2nT")
        for j in range(4):
            t = 4 * c + j
            st = stats.tile([P, 6], FP32)
            nc.vector.bn_stats(st, x2_sb[:, t, :])
            mv = stats.tile([P, 2], FP32)
            nc.vector.bn_aggr(mv, st)
            std = stats.tile([P, 1], FP32)
            nc.scalar.activation(std, mv[:, 1:2], AF.Sqrt, bias=eps_t)
            r2 = stats.tile([P, 1], FP32)
            nc.vector.reciprocal(r2, std)
            h2n = xnp.tile([P, D], BF16)
            nc.vector.tensor_scalar(h2n, x2_sb[:, t, :], scalar1=mv[:, 0:1], scalar2=r2,
                                    op0=ALU.subtract, op1=ALU.mult)
            for kt in range(KT):
                tp = ps_tp.tile([P, P], BF16, tag="pt")
                nc.tensor.transpose(tp, h2n[:, kt * P:(kt + 1) * P], id_bf)
                nc.vector.tensor_copy(h2nT[:, kt, j * P:(j + 1) * P], tp)
        GT = work.tile([P, FT, 512], BF16, tag="GT")
        for mt in range(FT):
            ps = ps_big.tile([P, 512], FP32, tag="pb")
            for kt in range(KT):
                nc.tensor.matmul(ps, lhsT=w1_l[:, kt, mt * P:(mt + 1) * P],
                                 rhs=h2nT[:, kt, :], start=(kt == 0), stop=(kt == KT - 1))
            nc.scalar.activation(GT[:, mt, :], ps, AF.Gelu, bias=b1c[:, mt:mt + 1])
        for j in range(4):
            t = 4 * c + j
            ps2 = ps_att.tile([P, D], FP32, tag="pa")
            for mt in range(FT):
                nc.tensor.matmul(ps2, lhsT=GT[:, mt, j * P:(j + 1) * P],
                                 rhs=w2_r[:, mt, :], start=(mt == 0), stop=False)
            nc.tensor.matmul(ps2, lhsT=ones_row, rhs=f2b_row, start=False, stop=True)
            o = work.tile([P, D], FP32, tag="out_t")
            nc.vector.tensor_add(o, ps2, x2_sb[:, t, :])
            nc.sync.dma_start(out=out_v[:, t, :], in_=o)


def build():
    nc = bacc.Bacc("TRN2", target_bir_lowering=False, debug=False)
    io = (
        nc.declare_dram_parameter("x", [N, D], FP32, isOutput=False),
        nc.declare_dram_parameter("g8", [1, N], FP32, isOutput=False),
        nc.declare_dram_parameter("wqk", [D, 4 * P], BF16, isOutput=False),
        nc.declare_dram_parameter("wv2", [D, D], BF16, isOutput=False),
        nc.declare_dram_parameter("batt", [1, D], BF16, isOutput=False),
        nc.declare_dram_parameter("tauw", [D, 1], BF16, isOutput=False),
        nc.declare_dram_parameter("taub6", [P, 1], FP32, isOutput=False),
        nc.declare_dram_parameter("w1", [D, DFF], BF16, isOutput=False),
        nc.declare_dram_parameter("b1c", [P, FT], FP32, isOutput=False),
        nc.declare_dram_parameter("w2", [DFF, D], BF16, isOutput=False),
        nc.declare_dram_parameter("f2b", [1, D], BF16, isOutput=False),
        nc.declare_dram_parameter("out", [N, D], FP32, isOutput=True),
        nc.declare_dram_parameter("attn", [N, N], FP32, isOutput=True),
    )
    with tile.TileContext(nc) as tc:
        with ExitStack() as ctx:
            _body(nc, tc, ctx, io)
    nc.compile()
    return nc


_NC = None


def _get_nc():
    global _NC
    if _NC is None:
        _NC = build()
    return _NC


def make_in_maps(**inputs):
    f = np.float32
    bf = ml_dtypes.bfloat16

    def a(k):
        return np.asarray(inputs[k], dtype=f)

    ln1_w, ln1_b = a("ln1_w").reshape(D), a("ln1_b").reshape(D)
    ln2_w, ln2_b = a("ln2_w").reshape(D), a("ln2_b").reshape(D)
    qkv_w, qkv_b = a("qkv_w"), a("qkv_b").reshape(3 * D)
    proj_w, proj_b = a("proj_w"), a("proj_b").reshape(D)
    tau_w, tau_b = a("tau_w").reshape(D, 1), a("tau_b").reshape(())
    f1w, f1b = a("ffn1_w"), a("ffn1_b").reshape(DFF)
    f2w, f2b = a("ffn2_w"), a("ffn2_b").reshape(D)

    w1f = ln1_w[:, None] * qkv_w          # LN1 scale fold
    wq = w1f[:, 0:D] / math.sqrt(D)
    wk = w1f[:, D:2 * D]
    wv = w1f[:, 2 * D:3 * D]
    wqk = np.concatenate([wq, wk], axis=1)              # [D, 512]
    wv2 = wv @ proj_w                                   # proj fold [D, D]
    bias_att = (ln1_b @ qkv_w[:, 2 * D:] + qkv_b[2 * D:]) @ proj_w + proj_b
    tauw = ln1_w[:, None] * tau_w
    taub12 = np.full((P, 1), -6.0 * (tau_b + float(ln1_b @ tau_w[:, 0])), dtype=f)
    w1 = ln2_w[:, None] * f1w
    b1 = ln2_b @ f1w + f1b                               # [DFF]
    b1c = b1.reshape(FT, P).T.copy()                     # [P, FT] col-major tiles
    g8 = np.asarray(inputs["g"], dtype=np.float64) ** 0.8

    shared = {
        "wqk": wqk.astype(bf),
        "wv2": wv2.astype(bf),
        "batt": bias_att.reshape(1, D).astype(bf),
        "tauw": tauw.reshape(D, 1).astype(bf),
        "taub6": taub12,
        "w1": w1.astype(bf),
        "b1c": np.ascontiguousarray(b1c, dtype=f),
        "w2": f2w.astype(bf),
        "f2b": f2b.reshape(1, D).astype(bf),
    }
    x = a("x")
    in_maps = []
    for b in range(B):
        m = dict(shared)
        m["x"] = np.ascontiguousarray(x[b])
        m["g8"] = np.ascontiguousarray(g8[b].reshape(1, N), dtype=f)
        in_maps.append(m)
    return in_maps


def kernel(**inputs):
    nc = _get_nc()
    res = run_bass_kernel_spmd(nc, make_in_maps(**inputs), core_ids=list(range(B)))
    out = np.stack([r["out"] for r in res.results]).astype(np.float32)
    attn = np.stack([r["attn"] for r in res.results]).astype(np.float32)
    return out, attn


# revision 25
# speedup vs baseline: 2.3284x; 2.3284x over previous
"""AGSM layer (gated sparse attention + FFN) on 8 TRN2 NeuronCores.

Data-parallel over the batch: B=8 batch elements -> 8 cores, one full
[N=2048, D=256] attention layer per core.  No collectives.

Host-side (numpy, O(D^2) marshalling): fold LN1/LN2 scales+biases into
the adjacent weight matrices, fold 1/sqrt(D) into Wq, fold proj_w into
Wv (W@(v@proj) == (W@v)@proj), precompute g^0.8.

Device (per core, reference semantics):
  xn  = (x - mean)/sqrt(var+eps)            per-row LN (affine folded away)
  qT,kT = wqk^T @ xn^T, gated by g^0.8 along the free axis
  A   = q'k'^T                               -> output #2 (f32)
  tau = xn @ tauw'
  mask= sigmoid(12(A - tau)) = 0.5 (1 + tanh(6(A - tau)))  (tanh shares the
        ACT table-set with exp; sigmoid does not; tanh kept in f32 because
        mask ~ 0 means tanh ~ -1 and bf16 would round the tail to exactly -1)
  P   = mask * 0.5*exp(A)    (softmax max-shift skipped: |A| <~ 1.5)
  att = (P @ v_ext) / rowsum(P)              v_ext = xn @ (Wv' proj) + bias row
  x2  = x + att
  out = x2 + W2^T gelu(W1'^T LN2(x2) + b1)   -> output #1
"""

import math
from contextlib import ExitStack

import ml_dtypes
import numpy as np

import concourse.bass as bass
import concourse.bacc as bacc
import concourse.tile as tile
import concourse.mybir as mybir
from concourse.bass_utils import run_bass_kernel_spmd
from concourse.masks import make_identity

B, N, D, DFF = 8, 2048, 256, 1024
P = 128
NT = N // P    # 16 query/key tiles
KT = D // P    # 2 d-tiles
FT = DFF // P  # 8 dff tiles
EPS = 1e-5
LN_HALF = math.log(0.5)

FP32 = mybir.dt.float32
BF16 = mybir.dt.bfloat16
AF = mybir.ActivationFunctionType
ALU = mybir.AluOpType


def _body(nc, tc, ctx, io):
    x_d, g8_d, wqk_d, wv2_d, batt_d, tauw_d, taub6_d, w1_d, b1c_d, w2_d, \
        f2b_d, out_d, attn_d = io

    consts = ctx.enter_context(tc.tile_pool(name="consts", bufs=1))
    stats = ctx.enter_context(tc.tile_pool(name="stats", bufs=6))
    work = ctx.enter_context(tc.tile_pool(name="work", bufs=2))
    xnp = ctx.enter_context(tc.tile_pool(name="xnp", bufs=3))
    ps_big = ctx.enter_context(tc.tile_pool(name="ps_big", bufs=3, space="PSUM"))
    ps_att = ctx.enter_context(tc.tile_pool(name="ps_att", bufs=2, space="PSUM"))
    ps_tp = ctx.enter_context(tc.tile_pool(name="ps_tp", bufs=2, space="PSUM"))

    # ---------------- constants / weights ----------------
    id_bf = consts.tile([P, P], BF16)
    make_identity(nc, id_bf)
    eps_t = consts.tile([P, 1], FP32)
    nc.vector.memset(eps_t, EPS)
    ones_row = consts.tile([1, P], BF16)
    nc.vector.memset(ones_row, 1.0)
    lnhalf_t = consts.tile([P, 1], FP32)
    nc.vector.memset(lnhalf_t, LN_HALF)

    wqk_l = consts.tile([P, KT, 4 * P], BF16)
    wv2_r = consts.tile([P, KT, D], BF16)
    w1_l = consts.tile([P, KT, DFF], BF16)
    for kt in range(KT):
        nc.sync.dma_start(out=wqk_l[:, kt, :], in_=wqk_d[kt * P:(kt + 1) * P, :])
        nc.sync.dma_start(out=wv2_r[:, kt, :], in_=wv2_d[kt * P:(kt + 1) * P, :])
        nc.sync.dma_start(out=w1_l[:, kt, :], in_=w1_d[kt * P:(kt + 1) * P, :])
    w2_r = consts.tile([P, FT, D], BF16)
    for mt in range(FT):
        nc.sync.dma_start(out=w2_r[:, mt, :], in_=w2_d[mt * P:(mt + 1) * P, :])
    tauw_c = consts.tile([P, KT], BF16)
    nc.sync.dma_start(out=tauw_c, in_=tauw_d.ap().rearrange("(k p) o -> p (k o)", p=P, k=KT))
    taub12 = consts.tile([P, 1], FP32)
    nc.sync.dma_start(out=taub12, in_=taub6_d.ap())
    b1c = consts.tile([P, FT], FP32)
    nc.sync.dma_start(out=b1c, in_=b1c_d.ap())
    bias_att = consts.tile([1, D], BF16)
    nc.sync.dma_start(out=bias_att, in_=batt_d.ap())
    f2b_row = consts.tile([1, D], BF16)
    nc.sync.dma_start(out=f2b_row, in_=f2b_d.ap())

    g_ap = g8_d.ap()
    g_bc = bass.AP(tensor=g_ap.tensor, offset=g_ap.offset, ap=[[0, P]] + list(g_ap.ap[1:]))
    gp = consts.tile([P, N], FP32)
    nc.sync.dma_start(out=gp, in_=g_bc)

    # ---------------- LN1 + transposed normalized input ----------------
    x_sb = consts.tile([P, NT, D], FP32)
    nc.sync.dma_start(out=x_sb, in_=x_d.ap().rearrange("(t p) d -> p t d", p=P))
    xnT = consts.tile([P, KT, N], BF16)
    mv1 = consts.tile([P, NT, 2], FP32)
    for t in range(NT):
        st = stats.tile([P, 6], FP32)
        nc.vector.bn_stats(st, x_sb[:, t, :])
        nc.vector.bn_aggr(mv1[:, t, :], st)
    std1 = consts.tile([P, NT], FP32)
    nc.scalar.activation(std1, mv1[:, :, 1], AF.Sqrt, bias=eps_t)
    r1a = consts.tile([P, NT], FP32)
    nc.vector.reciprocal(r1a, std1)
    for t in range(NT):
        xn = xnp.tile([P, D], BF16)
        nc.vector.tensor_scalar(xn, x_sb[:, t, :], scalar1=mv1[:, t, 0:1],
                                scalar2=r1a[:, t:t + 1], op0=ALU.subtract, op1=ALU.mult)
        for kt in range(KT):
            tp = ps_tp.tile([P, P], BF16, tag="pt")
            nc.tensor.transpose(tp, xn[:, kt * P:(kt + 1) * P], id_bf)
            nc.vector.tensor_copy(xnT[:, kt, t * P:(t + 1) * P], tp)

    # ---------------- q/k (gated, transposed), v'' (proj-folded), tau ----------------
    qkT = consts.tile([P, 2 * KT, N], BF16)   # rows: q d-tiles 0..1, k d-tiles 2..3
    for mc in range(2 * KT):
        for c in range(4):
            ps = ps_big.tile([P, 512], FP32, tag="pb")
            for kt in range(KT):
                nc.tensor.matmul(ps, lhsT=wqk_l[:, kt, mc * P:(mc + 1) * P],
                                 rhs=xnT[:, kt, c * 512:(c + 1) * 512],
                                 start=(kt == 0), stop=(kt == KT - 1))
            nc.vector.tensor_mul(qkT[:, mc, c * 512:(c + 1) * 512], ps,
                                 gp[:, c * 512:(c + 1) * 512])

    v_ext = consts.tile([P, NT, D], BF16)
    for mt in range(NT):
        ps = ps_att.tile([P, D], FP32, tag="pa")
        for kt in range(KT):
            nc.tensor.matmul(ps, lhsT=xnT[:, kt, mt * P:(mt + 1) * P],
                             rhs=wv2_r[:, kt, :], start=(kt == 0), stop=False)
        nc.tensor.matmul(ps, lhsT=ones_row, rhs=bias_att, start=False, stop=True)
        nc.scalar.copy(v_ext[:, mt, :], ps)

    tanh6 = consts.tile([P, NT], FP32)
    for t in range(NT):
        ps = ps_att.tile([P, 1], FP32, tag="pa")
        for kt in range(KT):
            nc.tensor.matmul(ps, lhsT=xnT[:, kt, t * P:(t + 1) * P],
                             rhs=tauw_c[:, kt:kt + 1], start=(kt == 0), stop=(kt == KT - 1))
        nc.vector.scalar_tensor_tensor(out=tanh6[:, t:t + 1], in0=ps, scalar=-6.0,
                                       in1=taub12, op0=ALU.mult, op1=ALU.add)

    # ---------------- attention loop ----------------
    x2_sb = consts.tile([P, NT, D], FP32)
    attn_v = attn_d.ap().rearrange("(t p) m -> p t m", p=P)
    for t in range(NT):
        A_sb = work.tile([P, N], FP32, tag="A_sb")
        for c in range(4):
            ps = ps_big.tile([P, 512], FP32, tag="pb")
            for kt in range(KT):
                nc.tensor.matmul(ps, lhsT=qkT[:, kt, t * P:(t + 1) * P],
                                 rhs=qkT[:, KT + kt, c * 512:(c + 1) * 512],
                                 start=(kt == 0), stop=(kt == KT - 1))
            if c < 2:
                nc.scalar.copy(A_sb[:, c * 512:(c + 1) * 512], ps)
            else:
                nc.vector.tensor_copy(A_sb[:, c * 512:(c + 1) * 512], ps)
        nc.sync.dma_start(out=attn_v[:, t, :], in_=A_sb)
        # P = sigmoid(12(A-tau)) * exp(A); sigmoid via 0.5(1+tanh(.)) -- tanh
        # shares the ACT table-set with exp.  Tanh output stays f32 (near -1
        # the bf16 tail would round to exactly -1).
        E = work.tile([P, N], BF16, tag="E")
        nc.scalar.activation(E, A_sb, AF.Exp, bias=lnhalf_t)   # 0.5*exp(A)
        Th = work.tile([P, N], FP32, tag="Th")
        nc.scalar.activation(Th, A_sb, AF.Tanh, scale=6.0, bias=tanh6[:, t:t + 1])
        Pt = work.tile([P, N], BF16, tag="Pt")
        Tsum = stats.tile([P, 1], FP32)
        nc.vector.scalar_tensor_tensor(out=Pt, in0=Th, scalar=1.0, in1=E,
                                       op0=ALU.add, op1=ALU.mult, accum_out=Tsum)
        # The tanh LUT flushes to exactly -1 below ~-8, so a fully-masked row
        # sums to 0; guard the divide like the reference's +1e-12.
        Tsafe = stats.tile([P, 1], FP32)
        nc.vector.tensor_scalar_add(Tsafe, Tsum, 1e-30)
        PT = work.tile([P, NT, P], BF16, tag="PT")
        for q in range(4):   # 4 transposes per PSUM bank, then one copy out
            tp = ps_tp.tile([P, 512], BF16, tag="pt")
            for j in range(4):
                mt = 4 * q + j
                nc.tensor.transpose(tp[:, j * P:(j + 1) * P],
                                    Pt[:, mt * P:(mt + 1) * P], id_bf)
            nc.vector.tensor_copy(PT[:, 4 * q:4 * q + 4, :], tp)
        rT = stats.tile([P, 1], FP32)
        nc.vector.reciprocal(rT, Tsafe)
        ps_o = ps_att.tile([P, D], FP32, tag="pa")
        for mt in range(NT):
            nc.tensor.matmul(ps_o, lhsT=PT[:, mt, :], rhs=v_ext[:, mt, :],
                             start=(mt == 0), stop=(mt == NT - 1))
        nc.vector.scalar_tensor_tensor(out=x2_sb[:, t, :], in0=ps_o, scalar=rT,
                                       in1=x_sb[:, t, :], op0=ALU.mult, op1=ALU.add)

    # ---------------- FFN ----------------
    out_v = out_d.ap().rearrange("(t p) d -> p t d", p=P)
    mv2 = consts.tile([P, NT, 2], FP32)
    for t in range(NT):
        st = stats.tile([P, 6], FP32)
        nc.vector.bn_stats(st, x2_sb[:, t, :])
        nc.vector.bn_aggr(mv2[:, t, :], st)
    std2 = consts.tile([P, NT], FP32)
    nc.scalar.activation(std2, mv2[:, :, 1], AF.Sqrt, bias=eps_t)
    r2a = consts.tile([P, NT], FP32)
    nc.vector.reciprocal(r2a, std2)
    for c in range(4):
        h2nT = work.tile([P, KT, 512], BF16, tag="h2nT")
        for j in range(4):
            t = 4 * c + j
            h2n = xnp.tile([P, D], BF16)
            nc.vector.tensor_scalar(h2n, x2_sb[:, t, :], scalar1=mv2[:, t, 0:1],
                                    scalar2=r2a[:, t:t + 1],
                                    op0=ALU.subtract, op1=ALU.mult)
            for kt in range(KT):
                tp = ps_tp.tile([P, P], BF16, tag="pt")
                nc.tensor.transpose(tp, h2n[:, kt * P:(kt + 1) * P], id_bf)
                nc.vector.tensor_copy(h2nT[:, kt, j * P:(j + 1) * P], tp)
        GT = work.tile([P, FT, 512], BF16, tag="GT")
        for mt in range(FT):
            ps = ps_big.tile([P, 512], FP32, tag="pb")
            for kt in range(KT):
                nc.tensor.matmul(ps, lhsT=w1_l[:, kt, mt * P:(mt + 1) * P],
                                 rhs=h2nT[:, kt, :], start=(kt == 0), stop=(kt == KT - 1))
            nc.scalar.activation(GT[:, mt, :], ps, AF.Gelu, bias=b1c[:, mt:mt + 1])
        for j in range(4):
            t = 4 * c + j
            ps2 = ps_att.tile([P, D], FP32, tag="pa")
            for mt in range(FT):
                nc.tensor.matmul(ps2, lhsT=GT[:, mt, j * P:(j + 1) * P],
                                 rhs=w2_r[:, mt, :], start=(mt == 0), stop=False)
            nc.tensor.matmul(ps2, lhsT=ones_row, rhs=f2b_row, start=False, stop=True)
            o = work.tile([P, D], FP32, tag="out_t")
            nc.vector.tensor_add(o, ps2, x2_sb[:, t, :])
            nc.sync.dma_start(out=out_v[:, t, :], in_=o)


def build():
    nc = bacc.Bacc("TRN2", target_bir_lowering=False, debug=False)
    io = (
        nc.declare_dram_parameter("x", [N, D], FP32, isOutput=False),
        nc.declare_dram_parameter("g8", [1, N], FP32, isOutput=False),
        nc.declare_dram_parameter("wqk", [D, 4 * P], BF16, isOutput=False),
        nc.declare_dram_parameter("wv2", [D, D], BF16, isOutput=False),
        nc.declare_dram_parameter("batt", [1, D], BF16, isOutput=False),
        nc.declare_dram_parameter("tauw", [D, 1], BF16, isOutput=False),
        nc.declare_dram_parameter("taub6", [P, 1], FP32, isOutput=False),
        nc.declare_dram_parameter("w1", [D, DFF], BF16, isOutput=False),
        nc.declare_dram_parameter("b1c", [P, FT], FP32, isOutput=False),
        nc.declare_dram_parameter("w2", [DFF, D], BF16, isOutput=False),
        nc.declare_dram_parameter("f2b", [1, D], BF16, isOutput=False),
        nc.declare_dram_parameter("out", [N, D], FP32, isOutput=True),
        nc.declare_dram_parameter("attn", [N, N], FP32, isOutput=True),
    )
    with tile.TileContext(nc) as tc:
        with ExitStack() as ctx:
            _body(nc, tc, ctx, io)
    nc.compile()
    return nc


_NC = None


def _get_nc():
    global _NC
    if _NC is None:
        _NC = build()
    return _NC


def make_in_maps(**inputs):
    f = np.float32
    bf = ml_dtypes.bfloat16

    def a(k):
        return np.asarray(inputs[k], dtype=f)

    ln1_w, ln1_b = a("ln1_w").reshape(D), a("ln1_b").reshape(D)
    ln2_w, ln2_b = a("ln2_w").reshape(D), a("ln2_b").reshape(D)
    qkv_w, qkv_b = a("qkv_w"), a("qkv_b").reshape(3 * D)
    proj_w, proj_b = a("proj_w"), a("proj_b").reshape(D)
    tau_w, tau_b = a("tau_w").reshape(D, 1), a("tau_b").reshape(())
    f1w, f1b = a("ffn1_w"), a("ffn1_b").reshape(DFF)
    f2w, f2b = a("ffn2_w"), a("ffn2_b").reshape(D)

    w1f = ln1_w[:, None] * qkv_w          # LN1 scale fold
    wq = w1f[:, 0:D] / math.sqrt(D)
    wk = w1f[:, D:2 * D]
    wv = w1f[:, 2 * D:3 * D]
    wqk = np.concatenate([wq, wk], axis=1)              # [D, 512]
    wv2 = wv @ proj_w                                   # proj fold [D, D]
    bias_att = (ln1_b @ qkv_w[:, 2 * D:] + qkv_b[2 * D:]) @ proj_w + proj_b
    tauw = ln1_w[:, None] * tau_w
    taub12 = np.full((P, 1), -6.0 * (tau_b + float(ln1_b @ tau_w[:, 0])), dtype=f)
    w1 = ln2_w[:, None] * f1w
    b1 = ln2_b @ f1w + f1b                               # [DFF]
    b1c = b1.reshape(FT, P).T.copy()                     # [P, FT] col-major tiles
    g8 = np.asarray(inputs["g"], dtype=np.float64) ** 0.8

    shared = {
        "wqk": wqk.astype(bf),
        "wv2": wv2.astype(bf),
        "batt": bias_att.reshape(1, D).astype(bf),
        "tauw": tauw.reshape(D, 1).astype(bf),
        "taub6": taub12,
        "w1": w1.astype(bf),
        "b1c": np.ascontiguousarray(b1c, dtype=f),
        "w2": f2w.astype(bf),
        "f2b": f2b.reshape(1, D).astype(bf),
    }
    x = a("x")
    in_maps = []
    for b in range(B):
        m = dict(shared)
        m["x"] = np.ascontiguousarray(x[b])
        m["g8"] = np.ascontiguousarray(g8[b].reshape(1, N), dtype=f)
        in_maps.append(m)
    return in_maps


def kernel(**inputs):
    nc = _get_nc()
    res = run_bass_kernel_spmd(nc, make_in_maps(**inputs), core_ids=list(range(B)))
    out = np.stack([r["out"] for r in res.results]).astype(np.float32)
    attn = np.stack([r["attn"] for r in res.results]).astype(np.float32)
    return out, attn


# revision 28
# speedup vs baseline: 2.5081x; 1.0772x over previous
"""AGSM layer (gated sparse attention + FFN) on 8 TRN2 NeuronCores.

Data-parallel over the batch: B=8 batch elements -> 8 cores, one full
[N=2048, D=256] attention layer per core.  No collectives.

Host-side (numpy, O(D^2) marshalling): fold LN1/LN2 scales+biases into
the adjacent weight matrices, fold 1/sqrt(D) into Wq, fold proj_w into
Wv (W@(v@proj) == (W@v)@proj), precompute g^0.8.

Device (per core, reference semantics):
  xn  = (x - mean)/sqrt(var+eps)            per-row LN (affine folded away)
  qT,kT = wqk^T @ xn^T, gated by g^0.8 along the free axis
  A   = q'k'^T                               -> output #2 (f32)
  tau = xn @ tauw'
  mask= sigmoid(12(A - tau)) = 0.5 (1 + tanh(6(A - tau)))  (tanh shares the
        ACT table-set with exp; sigmoid does not; tanh kept in f32 because
        mask ~ 0 means tanh ~ -1 and bf16 would round the tail to exactly -1)
  P   = mask * 0.5*exp(A)    (softmax max-shift skipped: |A| <~ 1.5)
  att = (P @ v_ext) / rowsum(P)              v_ext = xn @ (Wv' proj) + bias row
  x2  = x + att
  out = x2 + W2^T gelu(W1'^T LN2(x2) + b1)   -> output #1
"""

import math
from contextlib import ExitStack

import ml_dtypes
import numpy as np

import concourse.bass as bass
import concourse.bacc as bacc
import concourse.tile as tile
import concourse.mybir as mybir
from concourse.bass_utils import run_bass_kernel_spmd
from concourse.masks import make_identity

B, N, D, DFF = 8, 2048, 256, 1024
P = 128
NT = N // P    # 16 query/key tiles
KT = D // P    # 2 d-tiles
FT = DFF // P  # 8 dff tiles
EPS = 1e-5
LN_HALF = math.log(0.5)

FP32 = mybir.dt.float32
BF16 = mybir.dt.bfloat16
AF = mybir.ActivationFunctionType
ALU = mybir.AluOpType


def _body(nc, tc, ctx, io):
    x_d, g8_d, wqk_d, wv2_d, batt_d, tauw_d, taub6_d, w1_d, b1c_d, w2_d, \
        f2b_d, out_d, attn_d = io

    consts = ctx.enter_context(tc.tile_pool(name="consts", bufs=1))
    stats = ctx.enter_context(tc.tile_pool(name="stats", bufs=6))
    work = ctx.enter_context(tc.tile_pool(name="work", bufs=3))
    xnp = ctx.enter_context(tc.tile_pool(name="xnp", bufs=3))
    ps_big = ctx.enter_context(tc.tile_pool(name="ps_big", bufs=3, space="PSUM"))
    ps_att = ctx.enter_context(tc.tile_pool(name="ps_att", bufs=2, space="PSUM"))
    ps_tp = ctx.enter_context(tc.tile_pool(name="ps_tp", bufs=2, space="PSUM"))

    # ---------------- constants / weights ----------------
    id_bf = consts.tile([P, P], BF16)
    make_identity(nc, id_bf)
    eps_t = consts.tile([P, 1], FP32)
    nc.vector.memset(eps_t, EPS)
    ones_row = consts.tile([1, P], BF16)
    nc.vector.memset(ones_row, 1.0)
    lnhalf_t = consts.tile([P, 1], FP32)
    nc.vector.memset(lnhalf_t, LN_HALF)

    wqk_l = consts.tile([P, KT, 4 * P], BF16)
    wv2_r = consts.tile([P, KT, D], BF16)
    w1_l = consts.tile([P, KT, DFF], BF16)
    for kt in range(KT):
        nc.sync.dma_start(out=wqk_l[:, kt, :], in_=wqk_d[kt * P:(kt + 1) * P, :])
        nc.sync.dma_start(out=wv2_r[:, kt, :], in_=wv2_d[kt * P:(kt + 1) * P, :])
        nc.sync.dma_start(out=w1_l[:, kt, :], in_=w1_d[kt * P:(kt + 1) * P, :])
    w2_r = consts.tile([P, FT, D], BF16)
    for mt in range(FT):
        nc.sync.dma_start(out=w2_r[:, mt, :], in_=w2_d[mt * P:(mt + 1) * P, :])
    tauw_c = consts.tile([P, KT], BF16)
    nc.sync.dma_start(out=tauw_c, in_=tauw_d.ap().rearrange("(k p) o -> p (k o)", p=P, k=KT))
    taub12 = consts.tile([P, 1], FP32)
    nc.sync.dma_start(out=taub12, in_=taub6_d.ap())
    b1c = consts.tile([P, FT], FP32)
    nc.sync.dma_start(out=b1c, in_=b1c_d.ap())
    bias_att = consts.tile([1, D], BF16)
    nc.sync.dma_start(out=bias_att, in_=batt_d.ap())
    f2b_row = consts.tile([1, D], BF16)
    nc.sync.dma_start(out=f2b_row, in_=f2b_d.ap())

    g_ap = g8_d.ap()
    g_bc = bass.AP(tensor=g_ap.tensor, offset=g_ap.offset, ap=[[0, P]] + list(g_ap.ap[1:]))
    gp = consts.tile([P, N], FP32)
    nc.sync.dma_start(out=gp, in_=g_bc)

    # ---------------- LN1 + transposed normalized input ----------------
    x_sb = consts.tile([P, NT, D], FP32)
    x_v = x_d.ap().rearrange("(t p) d -> p t d", p=P)
    for c in range(4):
        nc.sync.dma_start(out=x_sb[:, 4 * c:4 * c + 4, :], in_=x_v[:, 4 * c:4 * c + 4, :])
    xnT = consts.tile([P, KT, N], BF16)
    mv1 = consts.tile([P, NT, 2], FP32)
    for t in range(NT):
        st = stats.tile([P, 6], FP32)
        nc.vector.bn_stats(st, x_sb[:, t, :])
        nc.vector.bn_aggr(mv1[:, t, :], st)
    std1 = consts.tile([P, NT], FP32)
    nc.scalar.activation(std1, mv1[:, :, 1], AF.Sqrt, bias=eps_t)
    r1a = consts.tile([P, NT], FP32)
    nc.vector.reciprocal(r1a, std1)
    for t in range(NT):
        xn = xnp.tile([P, D], BF16)
        nc.vector.tensor_scalar(xn, x_sb[:, t, :], scalar1=mv1[:, t, 0:1],
                                scalar2=r1a[:, t:t + 1], op0=ALU.subtract, op1=ALU.mult)
        for kt in range(KT):
            tp = ps_tp.tile([P, P], BF16, tag="pt")
            nc.tensor.transpose(tp, xn[:, kt * P:(kt + 1) * P], id_bf)
            nc.vector.tensor_copy(xnT[:, kt, t * P:(t + 1) * P], tp)

    # ---------------- q/k (gated, transposed), v'' (proj-folded), tau ----------------
    qkT = consts.tile([P, 2 * KT, N], BF16)   # rows: q d-tiles 0..1, k d-tiles 2..3
    for mc in range(2 * KT):
        for c in range(4):
            ps = ps_big.tile([P, 512], FP32, tag="pb")
            for kt in range(KT):
                nc.tensor.matmul(ps, lhsT=wqk_l[:, kt, mc * P:(mc + 1) * P],
                                 rhs=xnT[:, kt, c * 512:(c + 1) * 512],
                                 start=(kt == 0), stop=(kt == KT - 1))
            nc.vector.tensor_mul(qkT[:, mc, c * 512:(c + 1) * 512], ps,
                                 gp[:, c * 512:(c + 1) * 512])

    v_ext = consts.tile([P, NT, D], BF16)
    for mt in range(NT):
        ps = ps_att.tile([P, D], FP32, tag="pa")
        for kt in range(KT):
            nc.tensor.matmul(ps, lhsT=xnT[:, kt, mt * P:(mt + 1) * P],
                             rhs=wv2_r[:, kt, :], start=(kt == 0), stop=False)
        nc.tensor.matmul(ps, lhsT=ones_row, rhs=bias_att, start=False, stop=True)
        nc.scalar.copy(v_ext[:, mt, :], ps)

    tanh6 = consts.tile([P, NT], FP32)
    for t in range(NT):
        ps = ps_att.tile([P, 1], FP32, tag="pa")
        for kt in range(KT):
            nc.tensor.matmul(ps, lhsT=xnT[:, kt, t * P:(t + 1) * P],
                             rhs=tauw_c[:, kt:kt + 1], start=(kt == 0), stop=(kt == KT - 1))
        nc.vector.scalar_tensor_tensor(out=tanh6[:, t:t + 1], in0=ps, scalar=-6.0,
                                       in1=taub12, op0=ALU.mult, op1=ALU.add)

    # ---------------- attention loop ----------------
    x2_sb = consts.tile([P, NT, D], FP32)
    attn_v = attn_d.ap().rearrange("(t p) m -> p t m", p=P)
    for t in range(NT):
        A_sb = work.tile([P, N], FP32, tag="A_sb")
        for c in range(4):
            ps = ps_big.tile([P, 512], FP32, tag="pb")
            for kt in range(KT):
                nc.tensor.matmul(ps, lhsT=qkT[:, kt, t * P:(t + 1) * P],
                                 rhs=qkT[:, KT + kt, c * 512:(c + 1) * 512],
                                 start=(kt == 0), stop=(kt == KT - 1))
            if c < 3:
                nc.scalar.copy(A_sb[:, c * 512:(c + 1) * 512], ps)
            else:
                nc.vector.tensor_copy(A_sb[:, c * 512:(c + 1) * 512], ps)
        nc.sync.dma_start(out=attn_v[:, t, :], in_=A_sb)
        # P = sigmoid(12(A-tau)) * exp(A); sigmoid via 0.5(1+tanh(.)) -- tanh
        # shares the ACT table-set with exp.  Tanh output stays f32 (near -1
        # the bf16 tail would round to exactly -1).
        E = work.tile([P, N], BF16, tag="E")
        nc.scalar.activation(E, A_sb, AF.Exp, bias=lnhalf_t)   # 0.5*exp(A)
        Th = work.tile([P, N], FP32, tag="Th")
        nc.scalar.activation(Th, A_sb, AF.Tanh, scale=6.0, bias=tanh6[:, t:t + 1])
        Pt = work.tile([P, N], BF16, tag="Pt")
        Tsum = stats.tile([P, 1], FP32)
        nc.vector.scalar_tensor_tensor(out=Pt, in0=Th, scalar=1.0, in1=E,
                                       op0=ALU.add, op1=ALU.mult, accum_out=Tsum)
        # The tanh LUT flushes to exactly -1 below ~-8, so a fully-masked row
        # sums to 0; guard the divide like the reference's +1e-12.
        Tsafe = stats.tile([P, 1], FP32)
        nc.vector.tensor_scalar_add(Tsafe, Tsum, 1e-30)
        PT = work.tile([P, NT, P], BF16, tag="PT")
        for q in range(4):   # 4 transposes per PSUM bank, then one copy out
            tp = ps_tp.tile([P, 512], BF16, tag="pt")
            for j in range(4):
                mt = 4 * q + j
                nc.tensor.transpose(tp[:, j * P:(j + 1) * P],
                                    Pt[:, mt * P:(mt + 1) * P], id_bf)
            nc.vector.tensor_copy(PT[:, 4 * q:4 * q + 4, :], tp)
        rT = stats.tile([P, 1], FP32)
        nc.vector.reciprocal(rT, Tsafe)
        ps_o = ps_att.tile([P, D], FP32, tag="pa")
        for mt in range(NT):
            nc.tensor.matmul(ps_o, lhsT=PT[:, mt, :], rhs=v_ext[:, mt, :],
                             start=(mt == 0), stop=(mt == NT - 1))
        nc.vector.scalar_tensor_tensor(out=x2_sb[:, t, :], in0=ps_o, scalar=rT,
                                       in1=x_sb[:, t, :], op0=ALU.mult, op1=ALU.add)

    # ---------------- FFN ----------------
    out_v = out_d.ap().rearrange("(t p) d -> p t d", p=P)
    mv2 = consts.tile([P, NT, 2], FP32)
    for t in range(NT):
        st = stats.tile([P, 6], FP32)
        nc.vector.bn_stats(st, x2_sb[:, t, :])
        nc.vector.bn_aggr(mv2[:, t, :], st)
    std2 = consts.tile([P, NT], FP32)
    nc.scalar.activation(std2, mv2[:, :, 1], AF.Sqrt, bias=eps_t)
    r2a = consts.tile([P, NT], FP32)
    nc.vector.reciprocal(r2a, std2)
    for c in range(4):
        h2nT = work.tile([P, KT, 512], BF16, tag="h2nT")
        for j in range(4):
            t = 4 * c + j
            h2n = xnp.tile([P, D], BF16)
            nc.vector.tensor_scalar(h2n, x2_sb[:, t, :], scalar1=mv2[:, t, 0:1],
                                    scalar2=r2a[:, t:t + 1],
                                    op0=ALU.subtract, op1=ALU.mult)
            for kt in range(KT):
                tp = ps_tp.tile([P, P], BF16, tag="pt")
                nc.tensor.transpose(tp, h2n[:, kt * P:(kt + 1) * P], id_bf)
                nc.vector.tensor_copy(h2nT[:, kt, j * P:(j + 1) * P], tp)
        GT = work.tile([P, FT, 512], BF16, tag="GT")
        for mt in range(FT):
            ps = ps_big.tile([P, 512], FP32, tag="pb")
            for kt in range(KT):
                nc.tensor.matmul(ps, lhsT=w1_l[:, kt, mt * P:(mt + 1) * P],
                                 rhs=h2nT[:, kt, :], start=(kt == 0), stop=(kt == KT - 1))
            nc.scalar.activation(GT[:, mt, :], ps, AF.Gelu, bias=b1c[:, mt:mt + 1])
        for j in range(4):
            t = 4 * c + j
            ps2 = ps_att.tile([P, D], FP32, tag="pa")
            for mt in range(FT):
                nc.tensor.matmul(ps2, lhsT=GT[:, mt, j * P:(j + 1) * P],
                                 rhs=w2_r[:, mt, :], start=(mt == 0), stop=False)
            nc.tensor.matmul(ps2, lhsT=ones_row, rhs=f2b_row, start=False, stop=True)
            o = work.tile([P, D], FP32, tag="out_t")
            nc.vector.tensor_add(o, ps2, x2_sb[:, t, :])
            nc.sync.dma_start(out=out_v[:, t, :], in_=o)


def build():
    nc = bacc.Bacc("TRN2", target_bir_lowering=False, debug=False)
    io = (
        nc.declare_dram_parameter("x", [N, D], FP32, isOutput=False),
        nc.declare_dram_parameter("g8", [1, N], FP32, isOutput=False),
        nc.declare_dram_parameter("wqk", [D, 4 * P], BF16, isOutput=False),
        nc.declare_dram_parameter("wv2", [D, D], BF16, isOutput=False),
        nc.declare_dram_parameter("batt", [1, D], BF16, isOutput=False),
        nc.declare_dram_parameter("tauw", [D, 1], BF16, isOutput=False),
        nc.declare_dram_parameter("taub6", [P, 1], FP32, isOutput=False),
        nc.declare_dram_parameter("w1", [D, DFF], BF16, isOutput=False),
        nc.declare_dram_parameter("b1c", [P, FT], FP32, isOutput=False),
        nc.declare_dram_parameter("w2", [DFF, D], BF16, isOutput=False),
        nc.declare_dram_parameter("f2b", [1, D], BF16, isOutput=False),
        nc.declare_dram_parameter("out", [N, D], FP32, isOutput=True),
        nc.declare_dram_parameter("attn", [N, N], FP32, isOutput=True),
    )
    with tile.TileContext(nc) as tc:
        with ExitStack() as ctx:
            _body(nc, tc, ctx, io)
    nc.compile()
    return nc


_NC = None


def _get_nc():
    global _NC
    if _NC is None:
        _NC = build()
    return _NC


def make_in_maps(**inputs):
    f = np.float32
    bf = ml_dtypes.bfloat16

    def a(k):
        return np.asarray(inputs[k], dtype=f)

    ln1_w, ln1_b = a("ln1_w").reshape(D), a("ln1_b").reshape(D)
    ln2_w, ln2_b = a("ln2_w").reshape(D), a("ln2_b").reshape(D)
    qkv_w, qkv_b = a("qkv_w"), a("qkv_b").reshape(3 * D)
    proj_w, proj_b = a("proj_w"), a("proj_b").reshape(D)
    tau_w, tau_b = a("tau_w").reshape(D, 1), a("tau_b").reshape(())
    f1w, f1b = a("ffn1_w"), a("ffn1_b").reshape(DFF)
    f2w, f2b = a("ffn2_w"), a("ffn2_b").reshape(D)

    w1f = ln1_w[:, None] * qkv_w          # LN1 scale fold
    wq = w1f[:, 0:D] / math.sqrt(D)
    wk = w1f[:, D:2 * D]
    wv = w1f[:, 2 * D:3 * D]
    wqk = np.concatenate([wq, wk], axis=1)              # [D, 512]
    wv2 = wv @ proj_w                                   # proj fold [D, D]
    bias_att = (ln1_b @ qkv_w[:, 2 * D:] + qkv_b[2 * D:]) @ proj_w + proj_b
    tauw = ln1_w[:, None] * tau_w
    taub12 = np.full((P, 1), -6.0 * (tau_b + float(ln1_b @ tau_w[:, 0])), dtype=f)
    w1 = ln2_w[:, None] * f1w
    b1 = ln2_b @ f1w + f1b                               # [DFF]
    b1c = b1.reshape(FT, P).T.copy()                     # [P, FT] col-major tiles
    g8 = np.asarray(inputs["g"], dtype=np.float64) ** 0.8

    shared = {
        "wqk": wqk.astype(bf),
        "wv2": wv2.astype(bf),
        "batt": bias_att.reshape(1, D).astype(bf),
        "tauw": tauw.reshape(D, 1).astype(bf),
        "taub6": taub12,
        "w1": w1.astype(bf),
        "b1c": np.ascontiguousarray(b1c, dtype=f),
        "w2": f2w.astype(bf),
        "f2b": f2b.reshape(1, D).astype(bf),
    }
    x = a("x")
    in_maps = []
    for b in range(B):
        m = dict(shared)
        m["x"] = np.ascontiguousarray(x[b])
        m["g8"] = np.ascontiguousarray(g8[b].reshape(1, N), dtype=f)
        in_maps.append(m)
    return in_maps


def kernel(**inputs):
    nc = _get_nc()
    res = run_bass_kernel_spmd(nc, make_in_maps(**inputs), core_ids=list(range(B)))
    out = np.stack([r["out"] for r in res.results]).astype(np.float32)
    attn = np.stack([r["attn"] for r in res.results]).astype(np.float32)
    return out, attn


# revision 33
# speedup vs baseline: 2.7761x; 1.1069x over previous
"""AGSM layer (gated sparse attention + FFN) on 8 TRN2 NeuronCores.

Data-parallel over the batch: B=8 batch elements -> 8 cores, one full
[N=2048, D=256] attention layer per core.  No collectives.

Host-side (numpy, O(D^2) marshalling): fold LN1/LN2 scales+biases into
the adjacent weight matrices, fold 1/sqrt(D) into Wq, fold proj_w into
Wv (W@(v@proj) == (W@v)@proj), precompute g^0.8.

Device (per core, reference semantics):
  xn  = (x - mean)/sqrt(var+eps)            per-row LN (affine folded away)
  qT,kT = wqk^T @ xn^T, gated by g^0.8 along the free axis
  A   = q'k'^T                               -> output #2 (f32)
  tau = xn @ tauw'
  mask= sigmoid(12(A - tau)) = 0.5 (1 + tanh(6(A - tau)))  (tanh shares the
        ACT table-set with exp; sigmoid does not; tanh kept in f32 because
        mask ~ 0 means tanh ~ -1 and bf16 would round the tail to exactly -1)
  P   = mask * 0.5*exp(A)    (softmax max-shift skipped: |A| <~ 1.5)
  att = (P @ v_ext) / rowsum(P)              v_ext = xn @ (Wv' proj) + bias row
  x2  = x + att
  out = x2 + W2^T gelu(W1'^T LN2(x2) + b1)   -> output #1
"""

import math
from contextlib import ExitStack

import ml_dtypes
import numpy as np

import concourse.bass as bass
import concourse.bacc as bacc
import concourse.tile as tile
import concourse.mybir as mybir
from concourse.bass_utils import run_bass_kernel_spmd
from concourse.masks import make_identity

B, N, D, DFF = 8, 2048, 256, 1024
P = 128
NT = N // P    # 16 query/key tiles
KT = D // P    # 2 d-tiles
FT = DFF // P  # 8 dff tiles
EPS = 1e-5
LN_HALF = math.log(0.5)

FP32 = mybir.dt.float32
BF16 = mybir.dt.bfloat16
AF = mybir.ActivationFunctionType
ALU = mybir.AluOpType


def _body(nc, tc, ctx, io):
    x_d, g8_d, wqk_d, wv2_d, batt_d, tauw_d, taub6_d, w1_d, b1c_d, w2_d, \
        f2b_d, out_d, attn_d = io

    consts = ctx.enter_context(tc.tile_pool(name="consts", bufs=1))
    stats = ctx.enter_context(tc.tile_pool(name="stats", bufs=6))
    work = ctx.enter_context(tc.tile_pool(name="work", bufs=3))
    xnp = ctx.enter_context(tc.tile_pool(name="xnp", bufs=3))
    ps_big = ctx.enter_context(tc.tile_pool(name="ps_big", bufs=3, space="PSUM"))
    ps_att = ctx.enter_context(tc.tile_pool(name="ps_att", bufs=2, space="PSUM"))
    ps_tp = ctx.enter_context(tc.tile_pool(name="ps_tp", bufs=2, space="PSUM"))

    # ---------------- constants / weights ----------------
    id_bf = consts.tile([P, P], BF16)
    make_identity(nc, id_bf)
    eps_t = consts.tile([P, 1], FP32)
    nc.vector.memset(eps_t, EPS)
    ones_row = consts.tile([1, P], BF16)
    nc.vector.memset(ones_row, 1.0)
    lnhalf_t = consts.tile([P, 1], FP32)
    nc.vector.memset(lnhalf_t, LN_HALF)

    g_ap = g8_d.ap()
    g_bc = bass.AP(tensor=g_ap.tensor, offset=g_ap.offset, ap=[[0, P]] + list(g_ap.ap[1:]))
    gp = consts.tile([P, N], FP32)
    nc.sync.dma_start(out=gp, in_=g_bc)

    # ---------------- LN1 + transposed normalized input ----------------
    x_sb = consts.tile([P, NT, D], FP32)
    x_v = x_d.ap().rearrange("(t p) d -> p t d", p=P)
    for c in range(4):
        nc.sync.dma_start(out=x_sb[:, 4 * c:4 * c + 4, :], in_=x_v[:, 4 * c:4 * c + 4, :])
    xnT = consts.tile([P, KT, N], BF16)
    mv1 = consts.tile([P, NT, 2], FP32)
    std1 = consts.tile([P, NT], FP32)
    r1a = consts.tile([P, NT], FP32)
    for grp in range(4):
        for j in range(4):
            t = 4 * grp + j
            st = stats.tile([P, 6], FP32)
            nc.vector.bn_stats(st, x_sb[:, t, :])
            nc.vector.bn_aggr(mv1[:, t, :], st)
        g4 = slice(4 * grp, 4 * grp + 4)
        nc.scalar.activation(std1[:, g4], mv1[:, g4, 1], AF.Sqrt, bias=eps_t)
        nc.vector.reciprocal(r1a[:, g4], std1[:, g4])
        for j in range(4):
            t = 4 * grp + j
            xn = xnp.tile([P, D], BF16)
            nc.vector.tensor_scalar(xn, x_sb[:, t, :], scalar1=mv1[:, t, 0:1],
                                    scalar2=r1a[:, t:t + 1], op0=ALU.subtract, op1=ALU.mult)
            tp = ps_tp.tile([P, D], BF16, tag="pt")
            for kt in range(KT):
                nc.tensor.transpose(tp[:, kt * P:(kt + 1) * P], xn[:, kt * P:(kt + 1) * P], id_bf)
            nc.vector.tensor_copy(xnT[:, :, t * P:(t + 1) * P],
                                  tp.rearrange("p (k n) -> p k n", k=KT))

    wqk_l = consts.tile([P, KT, 4 * P], BF16)
    wv2_r = consts.tile([P, KT, D], BF16)
    w1_l = consts.tile([P, KT, DFF], BF16)
    for kt in range(KT):
        nc.sync.dma_start(out=wqk_l[:, kt, :], in_=wqk_d[kt * P:(kt + 1) * P, :])
        nc.sync.dma_start(out=wv2_r[:, kt, :], in_=wv2_d[kt * P:(kt + 1) * P, :])
        nc.sync.dma_start(out=w1_l[:, kt, :], in_=w1_d[kt * P:(kt + 1) * P, :])
    w2_r = consts.tile([P, FT, D], BF16)
    for mt in range(FT):
        nc.sync.dma_start(out=w2_r[:, mt, :], in_=w2_d[mt * P:(mt + 1) * P, :])
    tauw_c = consts.tile([P, KT], BF16)
    nc.sync.dma_start(out=tauw_c, in_=tauw_d.ap().rearrange("(k p) o -> p (k o)", p=P, k=KT))
    taub12 = consts.tile([P, 1], FP32)
    nc.sync.dma_start(out=taub12, in_=taub6_d.ap())
    b1c = consts.tile([P, FT], FP32)
    nc.sync.dma_start(out=b1c, in_=b1c_d.ap())
    bias_att = consts.tile([1, D], BF16)
    nc.sync.dma_start(out=bias_att, in_=batt_d.ap())
    f2b_row = consts.tile([1, D], BF16)
    nc.sync.dma_start(out=f2b_row, in_=f2b_d.ap())


    # ---------------- q/k (gated, transposed), v'' (proj-folded), tau ----------------
    qkT = consts.tile([P, 2 * KT, N], BF16)   # rows: q d-tiles 0..1, k d-tiles 2..3
    for mc in range(2 * KT):
        for c in range(4):
            ps = ps_big.tile([P, 512], FP32, tag="pb")
            for kt in range(KT):
                nc.tensor.matmul(ps, lhsT=wqk_l[:, kt, mc * P:(mc + 1) * P],
                                 rhs=xnT[:, kt, c * 512:(c + 1) * 512],
                                 start=(kt == 0), stop=(kt == KT - 1))
            nc.vector.tensor_mul(qkT[:, mc, c * 512:(c + 1) * 512], ps,
                                 gp[:, c * 512:(c + 1) * 512])

    v_ext = consts.tile([P, NT, D], BF16)
    for mt in range(NT):
        ps = ps_att.tile([P, D], FP32, tag="pa")
        for kt in range(KT):
            nc.tensor.matmul(ps, lhsT=xnT[:, kt, mt * P:(mt + 1) * P],
                             rhs=wv2_r[:, kt, :], start=(kt == 0), stop=False)
        nc.tensor.matmul(ps, lhsT=ones_row, rhs=bias_att, start=False, stop=True)
        nc.scalar.copy(v_ext[:, mt, :], ps)

    tanh6 = consts.tile([P, NT], FP32)
    for t in range(NT):
        ps = ps_att.tile([P, 1], FP32, tag="pa")
        for kt in range(KT):
            nc.tensor.matmul(ps, lhsT=xnT[:, kt, t * P:(t + 1) * P],
                             rhs=tauw_c[:, kt:kt + 1], start=(kt == 0), stop=(kt == KT - 1))
        nc.vector.scalar_tensor_tensor(out=tanh6[:, t:t + 1], in0=ps, scalar=-6.0,
                                       in1=taub12, op0=ALU.mult, op1=ALU.add)

    # ---------------- attention loop ----------------
    x2_sb = consts.tile([P, NT, D], FP32)
    attn_v = attn_d.ap().rearrange("(t p) m -> p t m", p=P)
    for t in range(NT):
        A_sb = work.tile([P, N], FP32, tag="A_sb")
        for c in range(4):
            ps = ps_big.tile([P, 512], FP32, tag="pb")
            for kt in range(KT):
                nc.tensor.matmul(ps, lhsT=qkT[:, kt, t * P:(t + 1) * P],
                                 rhs=qkT[:, KT + kt, c * 512:(c + 1) * 512],
                                 start=(kt == 0), stop=(kt == KT - 1))
            if c < 3:
                nc.scalar.copy(A_sb[:, c * 512:(c + 1) * 512], ps)
            else:
                nc.vector.tensor_copy(A_sb[:, c * 512:(c + 1) * 512], ps)
        nc.sync.dma_start(out=attn_v[:, t, :], in_=A_sb)
        # P = sigmoid(12(A-tau)) * exp(A); sigmoid via 0.5(1+tanh(.)) -- tanh
        # shares the ACT table-set with exp.  Tanh output stays f32 (near -1
        # the bf16 tail would round to exactly -1).
        E = work.tile([P, N], BF16, tag="E")
        nc.scalar.activation(E, A_sb, AF.Exp, bias=lnhalf_t)   # 0.5*exp(A)
        Th = work.tile([P, N], FP32, tag="Th")
        nc.scalar.activation(Th, A_sb, AF.Tanh, scale=6.0, bias=tanh6[:, t:t + 1])
        Pt = work.tile([P, N], BF16, tag="Pt")
        Tsum = stats.tile([P, 1], FP32)
        nc.vector.scalar_tensor_tensor(out=Pt, in0=Th, scalar=1.0, in1=E,
                                       op0=ALU.add, op1=ALU.mult, accum_out=Tsum)
        # The tanh LUT flushes to exactly -1 below ~-8, so a fully-masked row
        # sums to 0; guard the divide like the reference's +1e-12.
        Tsafe = stats.tile([P, 1], FP32)
        nc.vector.tensor_scalar_add(Tsafe, Tsum, 1e-30)
        PT = work.tile([P, NT, P], BF16, tag="PT")
        for q in range(2):   # 8 transposes per (bf16) PSUM bank, then one copy
            tp = ps_tp.tile([P, 1024], BF16, tag="pt")
            for j in range(8):
                mt = 8 * q + j
                nc.tensor.transpose(tp[:, j * P:(j + 1) * P],
                                    Pt[:, mt * P:(mt + 1) * P], id_bf)
            nc.vector.tensor_copy(PT[:, 8 * q:8 * q + 8, :],
                                  tp.rearrange("p (m n) -> p m n", m=8))
        rT = stats.tile([P, 1], FP32)
        nc.vector.reciprocal(rT, Tsafe)
        ps_o = ps_att.tile([P, D], FP32, tag="pa")
        for mt in range(NT):
            nc.tensor.matmul(ps_o, lhsT=PT[:, mt, :], rhs=v_ext[:, mt, :],
                             start=(mt == 0), stop=(mt == NT - 1))
        nc.vector.scalar_tensor_tensor(out=x2_sb[:, t, :], in0=ps_o, scalar=rT,
                                       in1=x_sb[:, t, :], op0=ALU.mult, op1=ALU.add)

    # ---------------- FFN ----------------
    out_v = out_d.ap().rearrange("(t p) d -> p t d", p=P)
    mv2 = consts.tile([P, NT, 2], FP32)
    for t in range(NT):
        st = stats.tile([P, 6], FP32)
        nc.vector.bn_stats(st, x2_sb[:, t, :])
        nc.vector.bn_aggr(mv2[:, t, :], st)
    std2 = consts.tile([P, NT], FP32)
    nc.scalar.activation(std2, mv2[:, :, 1], AF.Sqrt, bias=eps_t)
    r2a = consts.tile([P, NT], FP32)
    nc.vector.reciprocal(r2a, std2)
    for c in range(4):
        h2nT = work.tile([P, KT, 512], BF16, tag="h2nT")
        for j in range(4):
            t = 4 * c + j
            h2n = xnp.tile([P, D], BF16)
            nc.vector.tensor_scalar(h2n, x2_sb[:, t, :], scalar1=mv2[:, t, 0:1],
                                    scalar2=r2a[:, t:t + 1],
                                    op0=ALU.subtract, op1=ALU.mult)
            tp = ps_tp.tile([P, D], BF16, tag="pt")
            for kt in range(KT):
                nc.tensor.transpose(tp[:, kt * P:(kt + 1) * P],
                                    h2n[:, kt * P:(kt + 1) * P], id_bf)
            nc.vector.tensor_copy(h2nT[:, :, j * P:(j + 1) * P],
                                  tp.rearrange("p (k n) -> p k n", k=KT))
        GT = work.tile([P, FT, 512], BF16, tag="GT")
        for mt in range(FT):
            ps = ps_big.tile([P, 512], FP32, tag="pb")
            for kt in range(KT):
                nc.tensor.matmul(ps, lhsT=w1_l[:, kt, mt * P:(mt + 1) * P],
                                 rhs=h2nT[:, kt, :], start=(kt == 0), stop=(kt == KT - 1))
            nc.scalar.activation(GT[:, mt, :], ps, AF.Gelu, bias=b1c[:, mt:mt + 1])
        for j in range(4):
            t = 4 * c + j
            ps2 = ps_att.tile([P, D], FP32, tag="pa")
            for mt in range(FT):
                nc.tensor.matmul(ps2, lhsT=GT[:, mt, j * P:(j + 1) * P],
                                 rhs=w2_r[:, mt, :], start=(mt == 0), stop=False)
            nc.tensor.matmul(ps2, lhsT=ones_row, rhs=f2b_row, start=False, stop=True)
            o = work.tile([P, D], FP32, tag="out_t")
            nc.vector.tensor_add(o, ps2, x2_sb[:, t, :])
            nc.sync.dma_start(out=out_v[:, t, :], in_=o)


def build():
    nc = bacc.Bacc("TRN2", target_bir_lowering=False, debug=False)
    io = (
        nc.declare_dram_parameter("x", [N, D], FP32, isOutput=False),
        nc.declare_dram_parameter("g8", [1, N], FP32, isOutput=False),
        nc.declare_dram_parameter("wqk", [D, 4 * P], BF16, isOutput=False),
        nc.declare_dram_parameter("wv2", [D, D], BF16, isOutput=False),
        nc.declare_dram_parameter("batt", [1, D], BF16, isOutput=False),
        nc.declare_dram_parameter("tauw", [D, 1], BF16, isOutput=False),
        nc.declare_dram_parameter("taub6", [P, 1], FP32, isOutput=False),
        nc.declare_dram_parameter("w1", [D, DFF], BF16, isOutput=False),
        nc.declare_dram_parameter("b1c", [P, FT], FP32, isOutput=False),
        nc.declare_dram_parameter("w2", [DFF, D], BF16, isOutput=False),
        nc.declare_dram_parameter("f2b", [1, D], BF16, isOutput=False),
        nc.declare_dram_parameter("out", [N, D], FP32, isOutput=True),
        nc.declare_dram_parameter("attn", [N, N], FP32, isOutput=True),
    )
    with tile.TileContext(nc) as tc:
        with ExitStack() as ctx:
            _body(nc, tc, ctx, io)
    nc.compile()
    return nc


_NC = None


def _get_nc():
    global _NC
    if _NC is None:
        _NC = build()
    return _NC


def make_in_maps(**inputs):
    f = np.float32
    bf = ml_dtypes.bfloat16

    def a(k):
        return np.asarray(inputs[k], dtype=f)

    ln1_w, ln1_b = a("ln1_w").reshape(D), a("ln1_b").reshape(D)
    ln2_w, ln2_b = a("ln2_w").reshape(D), a("ln2_b").reshape(D)
    qkv_w, qkv_b = a("qkv_w"), a("qkv_b").reshape(3 * D)
    proj_w, proj_b = a("proj_w"), a("proj_b").reshape(D)
    tau_w, tau_b = a("tau_w").reshape(D, 1), a("tau_b").reshape(())
    f1w, f1b = a("ffn1_w"), a("ffn1_b").reshape(DFF)
    f2w, f2b = a("ffn2_w"), a("ffn2_b").reshape(D)

    w1f = ln1_w[:, None] * qkv_w          # LN1 scale fold
    wq = w1f[:, 0:D] / math.sqrt(D)
    wk = w1f[:, D:2 * D]
    wv = w1f[:, 2 * D:3 * D]
    wqk = np.concatenate([wq, wk], axis=1)              # [D, 512]
    wv2 = wv @ proj_w                                   # proj fold [D, D]
    bias_att = (ln1_b @ qkv_w[:, 2 * D:] + qkv_b[2 * D:]) @ proj_w + proj_b
    tauw = ln1_w[:, None] * tau_w
    taub12 = np.full((P, 1), -6.0 * (tau_b + float(ln1_b @ tau_w[:, 0])), dtype=f)
    w1 = ln2_w[:, None] * f1w
    b1 = ln2_b @ f1w + f1b                               # [DFF]
    b1c = b1.reshape(FT, P).T.copy()                     # [P, FT] col-major tiles
    g8 = np.asarray(inputs["g"], dtype=np.float64) ** 0.8

    shared = {
        "wqk": wqk.astype(bf),
        "wv2": wv2.astype(bf),
        "batt": bias_att.reshape(1, D).astype(bf),
        "tauw": tauw.reshape(D, 1).astype(bf),
        "taub6": taub12,
        "w1": w1.astype(bf),
        "b1c": np.ascontiguousarray(b1c, dtype=f),
        "w2": f2w.astype(bf),
        "f2b": f2b.reshape(1, D).astype(bf),
    }
    x = a("x")
    in_maps = []
    for b in range(B):
        m = dict(shared)
        m["x"] = np.ascontiguousarray(x[b])
        m["g8"] = np.ascontiguousarray(g8[b].reshape(1, N), dtype=f)
        in_maps.append(m)
    return in_maps


def kernel(**inputs):
    nc = _get_nc()
    res = run_bass_kernel_spmd(nc, make_in_maps(**inputs), core_ids=list(range(B)))
    out = np.stack([r["out"] for r in res.results]).astype(np.float32)
    attn = np.stack([r["attn"] for r in res.results]).astype(np.float32)
    return out, attn


# revision 34
# speedup vs baseline: 2.8253x; 1.0177x over previous
"""AGSM layer (gated sparse attention + FFN) on 8 TRN2 NeuronCores.

Data-parallel over the batch: B=8 batch elements -> 8 cores, one full
[N=2048, D=256] attention layer per core.  No collectives.

Host-side (numpy, O(D^2) marshalling): fold LN1/LN2 scales+biases into
the adjacent weight matrices, fold 1/sqrt(D) into Wq, fold proj_w into
Wv (W@(v@proj) == (W@v)@proj), precompute g^0.8.

Device (per core, reference semantics):
  xn  = (x - mean)/sqrt(var+eps)            per-row LN (affine folded away)
  qT,kT = wqk^T @ xn^T, gated by g^0.8 along the free axis
  A   = q'k'^T                               -> output #2 (f32)
  tau = xn @ tauw'
  mask= sigmoid(12(A - tau)) = 0.5 (1 + tanh(6(A - tau)))  (tanh shares the
        ACT table-set with exp; sigmoid does not; tanh kept in f32 because
        mask ~ 0 means tanh ~ -1 and bf16 would round the tail to exactly -1)
  P   = mask * 0.5*exp(A)    (softmax max-shift skipped: |A| <~ 1.5)
  att = (P @ v_ext) / rowsum(P)              v_ext = xn @ (Wv' proj) + bias row
  x2  = x + att
  out = x2 + W2^T gelu(W1'^T LN2(x2) + b1)   -> output #1
"""

import math
from contextlib import ExitStack

import ml_dtypes
import numpy as np

import concourse.bass as bass
import concourse.bacc as bacc
import concourse.tile as tile
import concourse.mybir as mybir
from concourse.bass_utils import run_bass_kernel_spmd
from concourse.masks import make_identity

B, N, D, DFF = 8, 2048, 256, 1024
P = 128
NT = N // P    # 16 query/key tiles
KT = D // P    # 2 d-tiles
FT = DFF // P  # 8 dff tiles
EPS = 1e-5
LN_HALF = math.log(0.5)

FP32 = mybir.dt.float32
BF16 = mybir.dt.bfloat16
FP16 = mybir.dt.float16
AF = mybir.ActivationFunctionType
ALU = mybir.AluOpType


def _body(nc, tc, ctx, io):
    x_d, g8_d, wqk_d, wv2_d, batt_d, tauw_d, taub6_d, w1_d, b1c_d, w2_d, \
        f2b_d, out_d, attn_d = io

    consts = ctx.enter_context(tc.tile_pool(name="consts", bufs=1))
    stats = ctx.enter_context(tc.tile_pool(name="stats", bufs=6))
    work = ctx.enter_context(tc.tile_pool(name="work", bufs=3))
    xnp = ctx.enter_context(tc.tile_pool(name="xnp", bufs=3))
    ps_big = ctx.enter_context(tc.tile_pool(name="ps_big", bufs=3, space="PSUM"))
    ps_att = ctx.enter_context(tc.tile_pool(name="ps_att", bufs=2, space="PSUM"))
    ps_tp = ctx.enter_context(tc.tile_pool(name="ps_tp", bufs=2, space="PSUM"))

    # ---------------- constants / weights ----------------
    id_bf = consts.tile([P, P], BF16)
    make_identity(nc, id_bf)
    id_f16 = consts.tile([P, P], FP16)
    make_identity(nc, id_f16)
    eps_t = consts.tile([P, 1], FP32)
    nc.vector.memset(eps_t, EPS)
    ones_row = consts.tile([1, P], BF16)
    nc.vector.memset(ones_row, 1.0)
    lnhalf_t = consts.tile([P, 1], FP32)
    nc.vector.memset(lnhalf_t, LN_HALF)

    g_ap = g8_d.ap()
    g_bc = bass.AP(tensor=g_ap.tensor, offset=g_ap.offset, ap=[[0, P]] + list(g_ap.ap[1:]))
    gp = consts.tile([P, N], FP32)
    nc.sync.dma_start(out=gp, in_=g_bc)

    # ---------------- LN1 + transposed normalized input ----------------
    x_sb = consts.tile([P, NT, D], FP32)
    x_v = x_d.ap().rearrange("(t p) d -> p t d", p=P)
    for c in range(4):
        nc.sync.dma_start(out=x_sb[:, 4 * c:4 * c + 4, :], in_=x_v[:, 4 * c:4 * c + 4, :])
    xnT = consts.tile([P, KT, N], BF16)
    mv1 = consts.tile([P, NT, 2], FP32)
    std1 = consts.tile([P, NT], FP32)
    r1a = consts.tile([P, NT], FP32)
    for grp in range(4):
        for j in range(4):
            t = 4 * grp + j
            st = stats.tile([P, 6], FP32)
            nc.vector.bn_stats(st, x_sb[:, t, :])
            nc.vector.bn_aggr(mv1[:, t, :], st)
        g4 = slice(4 * grp, 4 * grp + 4)
        nc.scalar.activation(std1[:, g4], mv1[:, g4, 1], AF.Sqrt, bias=eps_t)
        nc.vector.reciprocal(r1a[:, g4], std1[:, g4])
        for j in range(4):
            t = 4 * grp + j
            xn = xnp.tile([P, D], BF16)
            nc.vector.tensor_scalar(xn, x_sb[:, t, :], scalar1=mv1[:, t, 0:1],
                                    scalar2=r1a[:, t:t + 1], op0=ALU.subtract, op1=ALU.mult)
            tp = ps_tp.tile([P, D], BF16, tag="pt")
            for kt in range(KT):
                nc.tensor.transpose(tp[:, kt * P:(kt + 1) * P], xn[:, kt * P:(kt + 1) * P], id_bf)
            nc.vector.tensor_copy(xnT[:, :, t * P:(t + 1) * P],
                                  tp.rearrange("p (k n) -> p k n", k=KT))

    wqk_l = consts.tile([P, KT, 4 * P], BF16)
    wv2_r = consts.tile([P, KT, D], BF16)
    w1_l = consts.tile([P, KT, DFF], BF16)
    for kt in range(KT):
        nc.sync.dma_start(out=wqk_l[:, kt, :], in_=wqk_d[kt * P:(kt + 1) * P, :])
        nc.sync.dma_start(out=wv2_r[:, kt, :], in_=wv2_d[kt * P:(kt + 1) * P, :])
        nc.sync.dma_start(out=w1_l[:, kt, :], in_=w1_d[kt * P:(kt + 1) * P, :])
    w2_r = consts.tile([P, FT, D], BF16)
    for mt in range(FT):
        nc.sync.dma_start(out=w2_r[:, mt, :], in_=w2_d[mt * P:(mt + 1) * P, :])
    tauw_c = consts.tile([P, KT], BF16)
    nc.sync.dma_start(out=tauw_c, in_=tauw_d.ap().rearrange("(k p) o -> p (k o)", p=P, k=KT))
    taub12 = consts.tile([P, 1], FP32)
    nc.sync.dma_start(out=taub12, in_=taub6_d.ap())
    b1c = consts.tile([P, FT], FP32)
    nc.sync.dma_start(out=b1c, in_=b1c_d.ap())
    bias_att = consts.tile([1, D], BF16)
    nc.sync.dma_start(out=bias_att, in_=batt_d.ap())
    f2b_row = consts.tile([1, D], BF16)
    nc.sync.dma_start(out=f2b_row, in_=f2b_d.ap())


    # ---------------- q/k (gated, transposed), v'' (proj-folded), tau ----------------
    qkT = consts.tile([P, 2 * KT, N], BF16)   # rows: q d-tiles 0..1, k d-tiles 2..3
    for mc in range(2 * KT):
        for c in range(4):
            ps = ps_big.tile([P, 512], FP32, tag="pb")
            for kt in range(KT):
                nc.tensor.matmul(ps, lhsT=wqk_l[:, kt, mc * P:(mc + 1) * P],
                                 rhs=xnT[:, kt, c * 512:(c + 1) * 512],
                                 start=(kt == 0), stop=(kt == KT - 1))
            nc.vector.tensor_mul(qkT[:, mc, c * 512:(c + 1) * 512], ps,
                                 gp[:, c * 512:(c + 1) * 512])

    v_ext = consts.tile([P, NT, D], FP16)
    for mt in range(NT):
        ps = ps_att.tile([P, D], FP32, tag="pa")
        for kt in range(KT):
            nc.tensor.matmul(ps, lhsT=xnT[:, kt, mt * P:(mt + 1) * P],
                             rhs=wv2_r[:, kt, :], start=(kt == 0), stop=False)
        nc.tensor.matmul(ps, lhsT=ones_row, rhs=bias_att, start=False, stop=True)
        nc.scalar.copy(v_ext[:, mt, :], ps)

    tanh6 = consts.tile([P, NT], FP32)
    for t in range(NT):
        ps = ps_att.tile([P, 1], FP32, tag="pa")
        for kt in range(KT):
            nc.tensor.matmul(ps, lhsT=xnT[:, kt, t * P:(t + 1) * P],
                             rhs=tauw_c[:, kt:kt + 1], start=(kt == 0), stop=(kt == KT - 1))
        nc.vector.scalar_tensor_tensor(out=tanh6[:, t:t + 1], in0=ps, scalar=-6.0,
                                       in1=taub12, op0=ALU.mult, op1=ALU.add)

    # ---------------- attention loop ----------------
    x2_sb = consts.tile([P, NT, D], FP32)
    attn_v = attn_d.ap().rearrange("(t p) m -> p t m", p=P)
    for t in range(NT):
        A_sb = work.tile([P, N], FP32, tag="A_sb")
        for c in range(4):
            ps = ps_big.tile([P, 512], FP32, tag="pb")
            for kt in range(KT):
                nc.tensor.matmul(ps, lhsT=qkT[:, kt, t * P:(t + 1) * P],
                                 rhs=qkT[:, KT + kt, c * 512:(c + 1) * 512],
                                 start=(kt == 0), stop=(kt == KT - 1))
            if c < 2:
                nc.scalar.copy(A_sb[:, c * 512:(c + 1) * 512], ps)
            else:
                nc.vector.tensor_copy(A_sb[:, c * 512:(c + 1) * 512], ps)
        nc.sync.dma_start(out=attn_v[:, t, :], in_=A_sb)
        # P = sigmoid(12(A-tau)) * exp(A); sigmoid via 0.5(1+tanh(.)) -- tanh
        # shares the ACT table-set with exp.  Tanh output fp16: 2^-11 step near -1
        # keeps enough mask tail (bf16's 2^-9 does not); epsilon guards the rest.
        E = work.tile([P, N], FP16, tag="E")
        nc.scalar.activation(E, A_sb, AF.Exp, bias=lnhalf_t)   # 0.5*exp(A)
        Th = work.tile([P, N], FP16, tag="Th")
        nc.scalar.activation(Th, A_sb, AF.Tanh, scale=6.0, bias=tanh6[:, t:t + 1])
        Pt = work.tile([P, N], FP16, tag="Pt")
        Tsum = stats.tile([P, 1], FP32)
        nc.vector.scalar_tensor_tensor(out=Pt, in0=Th, scalar=1.0, in1=E,
                                       op0=ALU.add, op1=ALU.mult, accum_out=Tsum)
        # The tanh LUT flushes to exactly -1 below ~-8, so a fully-masked row
        # sums to 0; guard the divide like the reference's +1e-12.
        Tsafe = stats.tile([P, 1], FP32)
        nc.vector.tensor_scalar_add(Tsafe, Tsum, 1e-30)
        PT = work.tile([P, NT, P], FP16, tag="PT")
        for q in range(2):   # 8 transposes per (bf16) PSUM bank, then one copy
            tp = ps_tp.tile([P, 1024], FP16, tag="pt")
            for j in range(8):
                mt = 8 * q + j
                nc.tensor.transpose(tp[:, j * P:(j + 1) * P],
                                    Pt[:, mt * P:(mt + 1) * P], id_f16)
            nc.vector.tensor_copy(PT[:, 8 * q:8 * q + 8, :],
                                  tp.rearrange("p (m n) -> p m n", m=8))
        rT = stats.tile([P, 1], FP32)
        nc.vector.reciprocal(rT, Tsafe)
        ps_o = ps_att.tile([P, D], FP32, tag="pa")
        for mt in range(NT):
            nc.tensor.matmul(ps_o, lhsT=PT[:, mt, :], rhs=v_ext[:, mt, :],
                             start=(mt == 0), stop=(mt == NT - 1))
        nc.vector.scalar_tensor_tensor(out=x2_sb[:, t, :], in0=ps_o, scalar=rT,
                                       in1=x_sb[:, t, :], op0=ALU.mult, op1=ALU.add)

    # ---------------- FFN ----------------
    out_v = out_d.ap().rearrange("(t p) d -> p t d", p=P)
    mv2 = consts.tile([P, NT, 2], FP32)
    for t in range(NT):
        st = stats.tile([P, 6], FP32)
        nc.vector.bn_stats(st, x2_sb[:, t, :])
        nc.vector.bn_aggr(mv2[:, t, :], st)
    std2 = consts.tile([P, NT], FP32)
    nc.scalar.activation(std2, mv2[:, :, 1], AF.Sqrt, bias=eps_t)
    r2a = consts.tile([P, NT], FP32)
    nc.vector.reciprocal(r2a, std2)
    for c in range(4):
        h2nT = work.tile([P, KT, 512], BF16, tag="h2nT")
        for j in range(4):
            t = 4 * c + j
            h2n = xnp.tile([P, D], BF16)
            nc.vector.tensor_scalar(h2n, x2_sb[:, t, :], scalar1=mv2[:, t, 0:1],
                                    scalar2=r2a[:, t:t + 1],
                                    op0=ALU.subtract, op1=ALU.mult)
            tp = ps_tp.tile([P, D], BF16, tag="pt")
            for kt in range(KT):
                nc.tensor.transpose(tp[:, kt * P:(kt + 1) * P],
                                    h2n[:, kt * P:(kt + 1) * P], id_bf)
            nc.vector.tensor_copy(h2nT[:, :, j * P:(j + 1) * P],
                                  tp.rearrange("p (k n) -> p k n", k=KT))
        GT = work.tile([P, FT, 512], BF16, tag="GT")
        for mt in range(FT):
            ps = ps_big.tile([P, 512], FP32, tag="pb")
            for kt in range(KT):
                nc.tensor.matmul(ps, lhsT=w1_l[:, kt, mt * P:(mt + 1) * P],
                                 rhs=h2nT[:, kt, :], start=(kt == 0), stop=(kt == KT - 1))
            nc.scalar.activation(GT[:, mt, :], ps, AF.Gelu, bias=b1c[:, mt:mt + 1])
        for j in range(4):
            t = 4 * c + j
            ps2 = ps_att.tile([P, D], FP32, tag="pa")
            for mt in range(FT):
                nc.tensor.matmul(ps2, lhsT=GT[:, mt, j * P:(j + 1) * P],
                                 rhs=w2_r[:, mt, :], start=(mt == 0), stop=False)
            nc.tensor.matmul(ps2, lhsT=ones_row, rhs=f2b_row, start=False, stop=True)
            o = work.tile([P, D], FP32, tag="out_t")
            nc.vector.tensor_add(o, ps2, x2_sb[:, t, :])
            nc.sync.dma_start(out=out_v[:, t, :], in_=o)


def build():
    nc = bacc.Bacc("TRN2", target_bir_lowering=False, debug=False)
    io = (
        nc.declare_dram_parameter("x", [N, D], FP32, isOutput=False),
        nc.declare_dram_parameter("g8", [1, N], FP32, isOutput=False),
        nc.declare_dram_parameter("wqk", [D, 4 * P], BF16, isOutput=False),
        nc.declare_dram_parameter("wv2", [D, D], BF16, isOutput=False),
        nc.declare_dram_parameter("batt", [1, D], BF16, isOutput=False),
        nc.declare_dram_parameter("tauw", [D, 1], BF16, isOutput=False),
        nc.declare_dram_parameter("taub6", [P, 1], FP32, isOutput=False),
        nc.declare_dram_parameter("w1", [D, DFF], BF16, isOutput=False),
        nc.declare_dram_parameter("b1c", [P, FT], FP32, isOutput=False),
        nc.declare_dram_parameter("w2", [DFF, D], BF16, isOutput=False),
        nc.declare_dram_parameter("f2b", [1, D], BF16, isOutput=False),
        nc.declare_dram_parameter("out", [N, D], FP32, isOutput=True),
        nc.declare_dram_parameter("attn", [N, N], FP32, isOutput=True),
    )
    with tile.TileContext(nc) as tc:
        with ExitStack() as ctx:
            _body(nc, tc, ctx, io)
    nc.compile()
    return nc


_NC = None


def _get_nc():
    global _NC
    if _NC is None:
        _NC = build()
    return _NC


def make_in_maps(**inputs):
    f = np.float32
    bf = ml_dtypes.bfloat16

    def a(k):
        return np.asarray(inputs[k], dtype=f)

    ln1_w, ln1_b = a("ln1_w").reshape(D), a("ln1_b").reshape(D)
    ln2_w, ln2_b = a("ln2_w").reshape(D), a("ln2_b").reshape(D)
    qkv_w, qkv_b = a("qkv_w"), a("qkv_b").reshape(3 * D)
    proj_w, proj_b = a("proj_w"), a("proj_b").reshape(D)
    tau_w, tau_b = a("tau_w").reshape(D, 1), a("tau_b").reshape(())
    f1w, f1b = a("ffn1_w"), a("ffn1_b").reshape(DFF)
    f2w, f2b = a("ffn2_w"), a("ffn2_b").reshape(D)

    w1f = ln1_w[:, None] * qkv_w          # LN1 scale fold
    wq = w1f[:, 0:D] / math.sqrt(D)
    wk = w1f[:, D:2 * D]
    wv = w1f[:, 2 * D:3 * D]
    wqk = np.concatenate([wq, wk], axis=1)              # [D, 512]
    wv2 = wv @ proj_w                                   # proj fold [D, D]
    bias_att = (ln1_b @ qkv_w[:, 2 * D:] + qkv_b[2 * D:]) @ proj_w + proj_b
    tauw = ln1_w[:, None] * tau_w
    taub12 = np.full((P, 1), -6.0 * (tau_b + float(ln1_b @ tau_w[:, 0])), dtype=f)
    w1 = ln2_w[:, None] * f1w
    b1 = ln2_b @ f1w + f1b                               # [DFF]
    b1c = b1.reshape(FT, P).T.copy()                     # [P, FT] col-major tiles
    g8 = np.asarray(inputs["g"], dtype=np.float64) ** 0.8

    shared = {
        "wqk": wqk.astype(bf),
        "wv2": wv2.astype(bf),
        "batt": bias_att.reshape(1, D).astype(bf),
        "tauw": tauw.reshape(D, 1).astype(bf),
        "taub6": taub12,
        "w1": w1.astype(bf),
        "b1c": np.ascontiguousarray(b1c, dtype=f),
        "w2": f2w.astype(bf),
        "f2b": f2b.reshape(1, D).astype(bf),
    }
    x = a("x")
    in_maps = []
    for b in range(B):
        m = dict(shared)
        m["x"] = np.ascontiguousarray(x[b])
        m["g8"] = np.ascontiguousarray(g8[b].reshape(1, N), dtype=f)
        in_maps.append(m)
    return in_maps


def kernel(**inputs):
    nc = _get_nc()
    res = run_bass_kernel_spmd(nc, make_in_maps(**inputs), core_ids=list(range(B)))
    out = np.stack([r["out"] for r in res.results]).astype(np.float32)
    attn = np.stack([r["attn"] for r in res.results]).astype(np.float32)
    return out, attn


# revision 35
# speedup vs baseline: 2.8798x; 1.0193x over previous
"""AGSM layer (gated sparse attention + FFN) on 8 TRN2 NeuronCores.

Data-parallel over the batch: B=8 batch elements -> 8 cores, one full
[N=2048, D=256] attention layer per core.  No collectives.

Host-side (numpy, O(D^2) marshalling): fold LN1/LN2 scales+biases into
the adjacent weight matrices, fold 1/sqrt(D) into Wq, fold proj_w into
Wv (W@(v@proj) == (W@v)@proj), precompute g^0.8.

Device (per core, reference semantics):
  xn  = (x - mean)/sqrt(var+eps)            per-row LN (affine folded away)
  qT,kT = wqk^T @ xn^T, gated by g^0.8 along the free axis
  A   = q'k'^T                               -> output #2 (f32)
  tau = xn @ tauw'
  mask= sigmoid(12(A - tau)) = 0.5 (1 + tanh(6(A - tau)))  (tanh shares the
        ACT table-set with exp; sigmoid does not; tanh kept in f32 because
        mask ~ 0 means tanh ~ -1 and bf16 would round the tail to exactly -1)
  P   = mask * 0.5*exp(A)    (softmax max-shift skipped: |A| <~ 1.5)
  att = (P @ v_ext) / rowsum(P)              v_ext = xn @ (Wv' proj) + bias row
  x2  = x + att
  out = x2 + W2^T gelu(W1'^T LN2(x2) + b1)   -> output #1
"""

import math
from contextlib import ExitStack

import ml_dtypes
import numpy as np

import concourse.bass as bass
import concourse.bacc as bacc
import concourse.tile as tile
import concourse.mybir as mybir
from concourse.bass_utils import run_bass_kernel_spmd
from concourse.masks import make_identity

B, N, D, DFF = 8, 2048, 256, 1024
P = 128
NT = N // P    # 16 query/key tiles
KT = D // P    # 2 d-tiles
FT = DFF // P  # 8 dff tiles
EPS = 1e-5
LN_HALF = math.log(0.5)

FP32 = mybir.dt.float32
BF16 = mybir.dt.bfloat16
FP16 = mybir.dt.float16
AF = mybir.ActivationFunctionType
ALU = mybir.AluOpType


def _body(nc, tc, ctx, io):
    x_d, g8_d, wqk_d, wv2_d, batt_d, tauw_d, taub6_d, w1_d, b1c_d, w2_d, \
        f2b_d, out_d, attn_d = io

    consts = ctx.enter_context(tc.tile_pool(name="consts", bufs=1))
    stats = ctx.enter_context(tc.tile_pool(name="stats", bufs=6))
    work = ctx.enter_context(tc.tile_pool(name="work", bufs=3))
    xnp = ctx.enter_context(tc.tile_pool(name="xnp", bufs=3))
    ps_big = ctx.enter_context(tc.tile_pool(name="ps_big", bufs=3, space="PSUM"))
    ps_att = ctx.enter_context(tc.tile_pool(name="ps_att", bufs=2, space="PSUM"))
    ps_tp = ctx.enter_context(tc.tile_pool(name="ps_tp", bufs=2, space="PSUM"))

    # ---------------- constants / weights ----------------
    id_bf = consts.tile([P, P], BF16)
    make_identity(nc, id_bf)
    id_f16 = consts.tile([P, P], FP16)
    make_identity(nc, id_f16)
    eps_t = consts.tile([P, 1], FP32)
    nc.vector.memset(eps_t, EPS)
    ones_row = consts.tile([1, P], BF16)
    nc.vector.memset(ones_row, 1.0)
    lnhalf_t = consts.tile([P, 1], FP32)
    nc.vector.memset(lnhalf_t, LN_HALF)

    g_ap = g8_d.ap()
    g_bc = bass.AP(tensor=g_ap.tensor, offset=g_ap.offset, ap=[[0, P]] + list(g_ap.ap[1:]))
    gp = consts.tile([P, N], FP32)
    nc.sync.dma_start(out=gp, in_=g_bc)

    # ---------------- LN1 + transposed normalized input ----------------
    x_sb = consts.tile([P, NT, D], FP32)
    x_v = x_d.ap().rearrange("(t p) d -> p t d", p=P)
    for c in range(4):
        nc.sync.dma_start(out=x_sb[:, 4 * c:4 * c + 4, :], in_=x_v[:, 4 * c:4 * c + 4, :])
    xnT = consts.tile([P, KT, N], BF16)
    mv1 = consts.tile([P, NT, 2], FP32)
    std1 = consts.tile([P, NT], FP32)
    r1a = consts.tile([P, NT], FP32)
    for grp in range(4):
        for j in range(4):
            t = 4 * grp + j
            st = stats.tile([P, 6], FP32)
            nc.vector.bn_stats(st, x_sb[:, t, :])
            nc.vector.bn_aggr(mv1[:, t, :], st)
        g4 = slice(4 * grp, 4 * grp + 4)
        nc.scalar.activation(std1[:, g4], mv1[:, g4, 1], AF.Sqrt, bias=eps_t)
        nc.vector.reciprocal(r1a[:, g4], std1[:, g4])
        for j in range(4):
            t = 4 * grp + j
            xn = xnp.tile([P, D], BF16)
            nc.vector.tensor_scalar(xn, x_sb[:, t, :], scalar1=mv1[:, t, 0:1],
                                    scalar2=r1a[:, t:t + 1], op0=ALU.subtract, op1=ALU.mult)
            tp = ps_tp.tile([P, D], BF16, tag="pt")
            for kt in range(KT):
                nc.tensor.transpose(tp[:, kt * P:(kt + 1) * P], xn[:, kt * P:(kt + 1) * P], id_bf)
            nc.vector.tensor_copy(xnT[:, :, t * P:(t + 1) * P],
                                  tp.rearrange("p (k n) -> p k n", k=KT))

    wqk_l = consts.tile([P, KT, 4 * P], BF16)
    wv2_r = consts.tile([P, KT, D], BF16)
    w1_l = consts.tile([P, KT, DFF], BF16)
    for kt in range(KT):
        nc.sync.dma_start(out=wqk_l[:, kt, :], in_=wqk_d[kt * P:(kt + 1) * P, :])
        nc.sync.dma_start(out=wv2_r[:, kt, :], in_=wv2_d[kt * P:(kt + 1) * P, :])
        nc.sync.dma_start(out=w1_l[:, kt, :], in_=w1_d[kt * P:(kt + 1) * P, :])
    w2_r = consts.tile([P, FT, D], BF16)
    for mt in range(FT):
        nc.sync.dma_start(out=w2_r[:, mt, :], in_=w2_d[mt * P:(mt + 1) * P, :])
    tauw_c = consts.tile([P, KT], BF16)
    nc.sync.dma_start(out=tauw_c, in_=tauw_d.ap().rearrange("(k p) o -> p (k o)", p=P, k=KT))
    taub12 = consts.tile([P, 1], FP32)
    nc.sync.dma_start(out=taub12, in_=taub6_d.ap())
    b1c = consts.tile([P, FT], FP32)
    nc.sync.dma_start(out=b1c, in_=b1c_d.ap())
    bias_att = consts.tile([1, D], BF16)
    nc.sync.dma_start(out=bias_att, in_=batt_d.ap())
    f2b_row = consts.tile([1, D], BF16)
    nc.sync.dma_start(out=f2b_row, in_=f2b_d.ap())


    # ---------------- q/k (gated, transposed), v'' (proj-folded), tau ----------------
    qkT = consts.tile([P, 2 * KT, N], BF16)   # rows: q d-tiles 0..1, k d-tiles 2..3
    for mc in range(2 * KT):
        for c in range(4):
            ps = ps_big.tile([P, 512], FP32, tag="pb")
            for kt in range(KT):
                nc.tensor.matmul(ps, lhsT=wqk_l[:, kt, mc * P:(mc + 1) * P],
                                 rhs=xnT[:, kt, c * 512:(c + 1) * 512],
                                 start=(kt == 0), stop=(kt == KT - 1))
            nc.vector.tensor_mul(qkT[:, mc, c * 512:(c + 1) * 512], ps,
                                 gp[:, c * 512:(c + 1) * 512])

    v_ext = consts.tile([P, NT, D + 1], FP16)   # last column = 1: PV also row-sums P
    nc.vector.memset(v_ext[:, :, D:D + 1], 1.0)
    for mt in range(NT):
        ps = ps_att.tile([P, D], FP32, tag="pa")
        for kt in range(KT):
            nc.tensor.matmul(ps, lhsT=xnT[:, kt, mt * P:(mt + 1) * P],
                             rhs=wv2_r[:, kt, :], start=(kt == 0), stop=False)
        nc.tensor.matmul(ps, lhsT=ones_row, rhs=bias_att, start=False, stop=True)
        nc.scalar.copy(v_ext[:, mt, 0:D], ps)

    tanh6 = consts.tile([P, NT], FP32)
    for t in range(NT):
        ps = ps_att.tile([P, 1], FP32, tag="pa")
        for kt in range(KT):
            nc.tensor.matmul(ps, lhsT=xnT[:, kt, t * P:(t + 1) * P],
                             rhs=tauw_c[:, kt:kt + 1], start=(kt == 0), stop=(kt == KT - 1))
        nc.vector.scalar_tensor_tensor(out=tanh6[:, t:t + 1], in0=ps, scalar=-6.0,
                                       in1=taub12, op0=ALU.mult, op1=ALU.add)

    # ---------------- attention loop ----------------
    x2_sb = consts.tile([P, NT, D], FP32)
    attn_v = attn_d.ap().rearrange("(t p) m -> p t m", p=P)
    for t in range(NT):
        A_sb = work.tile([P, N], FP32, tag="A_sb")
        for c in range(4):
            ps = ps_big.tile([P, 512], FP32, tag="pb")
            for kt in range(KT):
                nc.tensor.matmul(ps, lhsT=qkT[:, kt, t * P:(t + 1) * P],
                                 rhs=qkT[:, KT + kt, c * 512:(c + 1) * 512],
                                 start=(kt == 0), stop=(kt == KT - 1))
            if c < 2:
                nc.scalar.copy(A_sb[:, c * 512:(c + 1) * 512], ps)
            else:
                nc.vector.tensor_copy(A_sb[:, c * 512:(c + 1) * 512], ps)
        nc.sync.dma_start(out=attn_v[:, t, :], in_=A_sb)
        # P = sigmoid(12(A-tau)) * exp(A); sigmoid via 0.5(1+tanh(.)) -- tanh
        # shares the ACT table-set with exp.  Tanh output fp16: 2^-11 step near -1
        # keeps enough mask tail (bf16's 2^-9 does not); epsilon guards the rest.
        E = work.tile([P, N], FP16, tag="E")
        nc.scalar.activation(E, A_sb, AF.Exp, bias=lnhalf_t)   # 0.5*exp(A)
        Th = work.tile([P, N], FP16, tag="Th")
        nc.scalar.activation(Th, A_sb, AF.Tanh, scale=6.0, bias=tanh6[:, t:t + 1])
        Thp = work.tile([P, N], FP16, tag="Thp")
        nc.vector.tensor_scalar_add(Thp, Th, 1.0)          # 2*sigmoid, fp16 4x
        Pt = work.tile([P, N], FP16, tag="Pt")
        nc.vector.tensor_mul(Pt, Thp, E)                   # fp16 2x
        PT = work.tile([P, NT, P], FP16, tag="PT")
        for q in range(2):   # 8 transposes per (bf16) PSUM bank, then one copy
            tp = ps_tp.tile([P, 1024], FP16, tag="pt")
            for j in range(8):
                mt = 8 * q + j
                nc.tensor.transpose(tp[:, j * P:(j + 1) * P],
                                    Pt[:, mt * P:(mt + 1) * P], id_f16)
            nc.vector.tensor_copy(PT[:, 8 * q:8 * q + 8, :],
                                  tp.rearrange("p (m n) -> p m n", m=8))
        ps_o = ps_att.tile([P, D + 1], FP32, tag="pa")
        for mt in range(NT):
            nc.tensor.matmul(ps_o, lhsT=PT[:, mt, :], rhs=v_ext[:, mt, :],
                             start=(mt == 0), stop=(mt == NT - 1))
        # column D holds T = rowsum(P); the tanh LUT flushes to exactly -1
        # below ~-8, so a fully-masked row sums to 0; guard the divide like
        # the reference's +1e-12.
        Tsafe = stats.tile([P, 1], FP32)
        nc.vector.tensor_scalar_add(Tsafe, ps_o[:, D:D + 1], 1e-30)
        rT = stats.tile([P, 1], FP32)
        nc.vector.reciprocal(rT, Tsafe)
        nc.vector.scalar_tensor_tensor(out=x2_sb[:, t, :], in0=ps_o[:, 0:D], scalar=rT,
                                       in1=x_sb[:, t, :], op0=ALU.mult, op1=ALU.add)

    # ---------------- FFN ----------------
    out_v = out_d.ap().rearrange("(t p) d -> p t d", p=P)
    mv2 = consts.tile([P, NT, 2], FP32)
    for t in range(NT):
        st = stats.tile([P, 6], FP32)
        nc.vector.bn_stats(st, x2_sb[:, t, :])
        nc.vector.bn_aggr(mv2[:, t, :], st)
    std2 = consts.tile([P, NT], FP32)
    nc.scalar.activation(std2, mv2[:, :, 1], AF.Sqrt, bias=eps_t)
    r2a = consts.tile([P, NT], FP32)
    nc.vector.reciprocal(r2a, std2)
    for c in range(4):
        h2nT = work.tile([P, KT, 512], BF16, tag="h2nT")
        for j in range(4):
            t = 4 * c + j
            h2n = xnp.tile([P, D], BF16)
            nc.vector.tensor_scalar(h2n, x2_sb[:, t, :], scalar1=mv2[:, t, 0:1],
                                    scalar2=r2a[:, t:t + 1],
                                    op0=ALU.subtract, op1=ALU.mult)
            tp = ps_tp.tile([P, D], BF16, tag="pt")
            for kt in range(KT):
                nc.tensor.transpose(tp[:, kt * P:(kt + 1) * P],
                                    h2n[:, kt * P:(kt + 1) * P], id_bf)
            nc.vector.tensor_copy(h2nT[:, :, j * P:(j + 1) * P],
                                  tp.rearrange("p (k n) -> p k n", k=KT))
        GT = work.tile([P, FT, 512], BF16, tag="GT")
        for mt in range(FT):
            ps = ps_big.tile([P, 512], FP32, tag="pb")
            for kt in range(KT):
                nc.tensor.matmul(ps, lhsT=w1_l[:, kt, mt * P:(mt + 1) * P],
                                 rhs=h2nT[:, kt, :], start=(kt == 0), stop=(kt == KT - 1))
            nc.scalar.activation(GT[:, mt, :], ps, AF.Gelu, bias=b1c[:, mt:mt + 1])
        for j in range(4):
            t = 4 * c + j
            ps2 = ps_att.tile([P, D], FP32, tag="pa")
            for mt in range(FT):
                nc.tensor.matmul(ps2, lhsT=GT[:, mt, j * P:(j + 1) * P],
                                 rhs=w2_r[:, mt, :], start=(mt == 0), stop=False)
            nc.tensor.matmul(ps2, lhsT=ones_row, rhs=f2b_row, start=False, stop=True)
            o = work.tile([P, D], FP32, tag="out_t")
            nc.vector.tensor_add(o, ps2, x2_sb[:, t, :])
            nc.sync.dma_start(out=out_v[:, t, :], in_=o)


def build():
    nc = bacc.Bacc("TRN2", target_bir_lowering=False, debug=False)
    io = (
        nc.declare_dram_parameter("x", [N, D], FP32, isOutput=False),
        nc.declare_dram_parameter("g8", [1, N], FP32, isOutput=False),
        nc.declare_dram_parameter("wqk", [D, 4 * P], BF16, isOutput=False),
        nc.declare_dram_parameter("wv2", [D, D], BF16, isOutput=False),
        nc.declare_dram_parameter("batt", [1, D], BF16, isOutput=False),
        nc.declare_dram_parameter("tauw", [D, 1], BF16, isOutput=False),
        nc.declare_dram_parameter("taub6", [P, 1], FP32, isOutput=False),
        nc.declare_dram_parameter("w1", [D, DFF], BF16, isOutput=False),
        nc.declare_dram_parameter("b1c", [P, FT], FP32, isOutput=False),
        nc.declare_dram_parameter("w2", [DFF, D], BF16, isOutput=False),
        nc.declare_dram_parameter("f2b", [1, D], BF16, isOutput=False),
        nc.declare_dram_parameter("out", [N, D], FP32, isOutput=True),
        nc.declare_dram_parameter("attn", [N, N], FP32, isOutput=True),
    )
    with tile.TileContext(nc) as tc:
        with ExitStack() as ctx:
            _body(nc, tc, ctx, io)
    nc.compile()
    return nc


_NC = None


def _get_nc():
    global _NC
    if _NC is None:
        _NC = build()
    return _NC


def make_in_maps(**inputs):
    f = np.float32
    bf = ml_dtypes.bfloat16

    def a(k):
        return np.asarray(inputs[k], dtype=f)

    ln1_w, ln1_b = a("ln1_w").reshape(D), a("ln1_b").reshape(D)
    ln2_w, ln2_b = a("ln2_w").reshape(D), a("ln2_b").reshape(D)
    qkv_w, qkv_b = a("qkv_w"), a("qkv_b").reshape(3 * D)
    proj_w, proj_b = a("proj_w"), a("proj_b").reshape(D)
    tau_w, tau_b = a("tau_w").reshape(D, 1), a("tau_b").reshape(())
    f1w, f1b = a("ffn1_w"), a("ffn1_b").reshape(DFF)
    f2w, f2b = a("ffn2_w"), a("ffn2_b").reshape(D)

    w1f = ln1_w[:, None] * qkv_w          # LN1 scale fold
    wq = w1f[:, 0:D] / math.sqrt(D)
    wk = w1f[:, D:2 * D]
    wv = w1f[:, 2 * D:3 * D]
    wqk = np.concatenate([wq, wk], axis=1)              # [D, 512]
    wv2 = wv @ proj_w                                   # proj fold [D, D]
    bias_att = (ln1_b @ qkv_w[:, 2 * D:] + qkv_b[2 * D:]) @ proj_w + proj_b
    tauw = ln1_w[:, None] * tau_w
    taub12 = np.full((P, 1), -6.0 * (tau_b + float(ln1_b @ tau_w[:, 0])), dtype=f)
    w1 = ln2_w[:, None] * f1w
    b1 = ln2_b @ f1w + f1b                               # [DFF]
    b1c = b1.reshape(FT, P).T.copy()                     # [P, FT] col-major tiles
    g8 = np.asarray(inputs["g"], dtype=np.float64) ** 0.8

    shared = {
        "wqk": wqk.astype(bf),
        "wv2": wv2.astype(bf),
        "batt": bias_att.reshape(1, D).astype(bf),
        "tauw": tauw.reshape(D, 1).astype(bf),
        "taub6": taub12,
        "w1": w1.astype(bf),
        "b1c": np.ascontiguousarray(b1c, dtype=f),
        "w2": f2w.astype(bf),
        "f2b": f2b.reshape(1, D).astype(bf),
    }
    x = a("x")
    in_maps = []
    for b in range(B):
        m = dict(shared)
        m["x"] = np.ascontiguousarray(x[b])
        m["g8"] = np.ascontiguousarray(g8[b].reshape(1, N), dtype=f)
        in_maps.append(m)
    return in_maps


def kernel(**inputs):
    nc = _get_nc()
    res = run_bass_kernel_spmd(nc, make_in_maps(**inputs), core_ids=list(range(B)))
    out = np.stack([r["out"] for r in res.results]).astype(np.float32)
    attn = np.stack([r["attn"] for r in res.results]).astype(np.float32)
    return out, attn
